# revision 8
# baseline (speedup 1.0000x reference)
"""Bicubic grid_sample (transpose-like warp) for Trainium2, 8 NeuronCores.

Strategy: shard output rows across cores (256 rows/core). The warp maps
output (i, j) -> input (y ~ j +- 21, x ~ i +- 21), so each core needs an
x-column slab of the image. On device, repack the slab into a patch table
in DRAM where each 256B unit holds the full 4x4x8ch bicubic patch at
(y0, x0) (fp16). v2: the table is built from a fully s,r-materialized
SBUF staging buffer and written via SWDGE with ~22KB contiguous
descriptors (16-engine spread); indices are folded into the gather's
wrapped 16-partition layout on-chip (no DRAM bounce); weights are
computed on 512-wide super-tiles; row-group 1's table build is emitted
interleaved with row-group 0's gather/combine so they overlap.
"""
import os, sys, types
sys.path.insert(0, "/opt/trn_rl_repo")
import numpy as np

try:  # register NTFF profile hook so BASS_TRACE=1 can measure HW time
    import antenv
    if "antenv.axon_hooks" not in sys.modules:
        from trn_agent_boot.trn_boot import _ntff_profile_via_ctypes
        _h = _ntff_profile_via_ctypes("/opt/axon/libaxon_pjrt.so")
        _m = types.ModuleType("antenv.axon_hooks")
        _m.get_axon_ntff_profile_hook = lambda: _h
        _m.set_axon_ntff_profile_hook = lambda h: None
        sys.modules["antenv.axon_hooks"] = _m
        antenv.axon_hooks = _m
except Exception:
    pass

import concourse.bass as bass
import concourse.bacc as bacc
import concourse.mybir as mybir
import concourse.tile as tile
from concourse import library_config
from concourse.bass_utils import run_bass_kernel_spmd

F32 = mybir.dt.float32
F16 = mybir.dt.float16
I16 = mybir.dt.int16
I32 = mybir.dt.int32
OP = mybir.AluOpType
ACTF = None  # set after import

N_CORES = 8
H = W = 2048
C = 8
RPC = H // N_CORES          # output rows per core = 256
PAD = 24                    # y halo rows on each side
YS = H + 2 * PAD            # 2096 slab rows
XS = 308                    # slab cols: [I0-24, I0+284)
XT = 176                    # table cols per row-group
XH = 88                     # x-half of the table staging buffer
YT = YS + 16                # table rows incl. pad so in_ap window stays in-bounds
SJW = 512                   # super-tile width (weights/idx granularity)
JW2 = 64                    # half-tile width (gather/combine granularity)
A = -0.75                   # bicubic constant
YB = 124                    # y-block rows for table build
N_YB = (YS + YB - 1) // YB  # 17


def build_nc():
    nc = bacc.Bacc("TRN2", target_bir_lowering=False, debug=False,
                   num_devices=N_CORES, num_swdge_queues=4)
    xs = nc.dram_tensor("xs", [C, YS + 4, XS], F32, kind="ExternalInput")
    gr = nc.dram_tensor("gr", [RPC, W, 2], F32, kind="ExternalInput")
    out = nc.dram_tensor("out", [C, RPC, W], F32, kind="ExternalOutput")

    with tile.TileContext(nc) as tc:
        nc.gpsimd.load_library(library_config.mlp)
        import contextlib
        with contextlib.ExitStack() as ctx:
            _build_body(ctx, tc, nc, xs, gr, out)
    nc.compile()
    return nc


def _build_body(ctx, tc, nc, xs, gr, out):
    Copy = mybir.ActivationFunctionType.Copy
    tabpool = ctx.enter_context(tc.tile_pool(name="tab", bufs=1, space="DRAM"))
    # phase-1 pools
    tpool = ctx.enter_context(tc.tile_pool(name="t", bufs=1))
    tgpool = ctx.enter_context(tc.tile_pool(name="tg", bufs=1))
    tg2pool = ctx.enter_context(tc.tile_pool(name="tg2", bufs=1))
    # phase-2 pools
    gridp = ctx.enter_context(tc.tile_pool(name="grid", bufs=2))
    wrk = ctx.enter_context(tc.tile_pool(name="wrk", bufs=1))
    wpp = ctx.enter_context(tc.tile_pool(name="wpp", bufs=2))
    idxp = ctx.enter_context(tc.tile_pool(name="idx", bufs=2))
    idxs1 = ctx.enter_context(tc.tile_pool(name="idx1", bufs=1))
    gp = ctx.enter_context(tc.tile_pool(name="g", bufs=2))
    lp = ctx.enter_context(tc.tile_pool(name="l", bufs=1))
    outp = ctx.enter_context(tc.tile_pool(name="out", bufs=2))

    tabs = []
    for g in range(2):
        tabg = tabpool.tile([YT * XT, 128], F16, tag=f"tab{g}")
        tabs.append(tabg)

    hwdge = [nc.sync, nc.scalar]
    cnt = {"dma": 0, "cp": 0, "q": 0}

    def eng():
        cnt["dma"] += 1
        return hwdge[cnt["dma"] % 2]

    def ccopy(dst, src):
        cnt["cp"] += 1
        if cnt["cp"] % 2 == 0:
            nc.vector.tensor_copy(dst, src)
        else:
            nc.scalar.copy(dst, src)

    # ---------------- phase 1: repack xs -> table[g], one y-block ----------
    def build_block(g, yb):
        y0 = yb * YB
        rows = min(YB, YS - y0)
        tg = tgpool.tile([128, 179 * 32], F16, tag="tg")
        for r in range(4):
            # one DMA for all 8 channels of row-shift r: t[p, c*179 + x]
            t = tpool.tile([128, 8 * 179], F32, tag=f"xsb{r % 2}")
            eng().dma_start(
                bass.AP(t.tensor, t.offset, [[t.ap[0][0], rows], [1, 8 * 179]]),
                bass.AP(xs, (y0 + r) * XS + 128 * g,
                        [[XS, rows], [(YS + 4) * XS, 8], [1, 179]]))
            # one interleave+cast op: tg[p, xu*32 + r*8 + c] = t[p, c*179+xu]
            dst = bass.AP(tg.tensor, tg.offset + r * 8,
                          [[tg.ap[0][0], rows], [1, 8], [32, 179]])
            srcap = bass.AP(t.tensor, t.offset,
                            [[t.ap[0][0], rows], [179, 8], [1, 179]])
            ccopy(dst, srcap)
        for h in range(2):
            tg2 = tg2pool.tile([128, XH * 128], F16, tag=f"tg2{h}")
            for s in range(4):
                src = bass.AP(tg.tensor, tg.offset + (XH * h + s) * 32,
                              [[tg.ap[0][0], rows], [32, XH], [1, 32]])
                dst = bass.AP(tg2.tensor, tg2.offset + s * 32,
                              [[tg2.ap[0][0], rows], [128, XH], [1, 32]])
                ccopy(dst, src)
            # one HWDGE DMA, contiguous 22.5KB per row on both sides
            dsta = bass.AP(tabs[g].tensor,
                           tabs[g].offset + (y0 * XT + h * XH) * 128,
                           [[XT * 128, rows], [1, XH * 128]])
            eng().dma_start(dsta, tg2[:rows, :])

    # ---------------- phase 2: per super-tile weights+idx, gather+combine --
    def cubic(t, tag, outdt):
        # returns w0..w3 tiles [128, SJW] in outdt; scratch tags shared
        # between calls (sequential use).
        s0 = wrk.tile([128, SJW], F32, tag="c_s0")
        nc.scalar.activation(s0[:], t[:], Copy, bias=1.0, scale=1.0)
        w0f = wrk.tile([128, SJW], F32, tag="c_w0f")
        nc.scalar.activation(w0f[:], s0[:], Copy, bias=-5.0 * A, scale=A)
        nc.vector.tensor_tensor(w0f[:], w0f[:], s0[:], op=OP.mult)
        nc.scalar.activation(w0f[:], w0f[:], Copy, bias=8.0 * A, scale=1.0)
        nc.vector.tensor_tensor(w0f[:], w0f[:], s0[:], op=OP.mult)
        w0 = wrk.tile([128, SJW], outdt, tag=f"w0{tag}")
        nc.scalar.activation(w0[:], w0f[:], Copy, bias=-4.0 * A, scale=1.0)
        # w1
        w1f = wrk.tile([128, SJW], F32, tag="c_w1f")
        nc.scalar.activation(w1f[:], t[:], Copy, bias=-(A + 3.0), scale=A + 2.0)
        t2 = wrk.tile([128, SJW], F32, tag="c_t2")
        nc.vector.tensor_tensor(t2[:], t[:], t[:], op=OP.mult)
        nc.vector.tensor_tensor(w1f[:], w1f[:], t2[:], op=OP.mult)
        w1 = wrk.tile([128, SJW], outdt, tag=f"w1{tag}")
        nc.scalar.activation(w1[:], w1f[:], Copy, bias=1.0, scale=1.0)
        # w2: u = 1 - t
        u = wrk.tile([128, SJW], F32, tag="c_u")
        nc.scalar.activation(u[:], t[:], Copy, bias=1.0, scale=-1.0)
        w2f = wrk.tile([128, SJW], F32, tag="c_w2f")
        nc.scalar.activation(w2f[:], u[:], Copy, bias=-(A + 3.0), scale=A + 2.0)
        u2 = wrk.tile([128, SJW], F32, tag="c_u2")
        nc.vector.tensor_tensor(u2[:], u[:], u[:], op=OP.mult)
        nc.vector.tensor_tensor(w2f[:], w2f[:], u2[:], op=OP.mult)
        w2 = wrk.tile([128, SJW], outdt, tag=f"w2{tag}")
        nc.scalar.activation(w2[:], w2f[:], Copy, bias=1.0, scale=1.0)
        # w3 = 1 - w0 - w1 - w2 (in f32 then cast)
        w3f = wrk.tile([128, SJW], F32, tag="c_w3f")
        nc.vector.tensor_tensor(w3f[:], w0[:], w1[:], op=OP.add)
        nc.vector.tensor_tensor(w3f[:], w3f[:], w2[:], op=OP.add)
        w3 = wrk.tile([128, SJW], outdt, tag=f"w3{tag}")
        nc.scalar.activation(w3[:], w3f[:], Copy, bias=1.0, scale=-1.0)
        return [w0, w1, w2, w3]

    def floorpair(v, tag):
        # vi/co scratch shared between calls; vf/fr persist per-dir
        vi = wrk.tile([128, SJW], I32, tag="f_vi")
        nc.vector.tensor_copy(vi[:], v[:])
        vf = wrk.tile([128, SJW], F32, tag=f"vf{tag}")
        nc.vector.tensor_copy(vf[:], vi[:])
        co = wrk.tile([128, SJW], F32, tag="f_co")
        nc.vector.tensor_tensor(co[:], vf[:], v[:], op=OP.is_gt)
        nc.vector.tensor_tensor(vf[:], vf[:], co[:], op=OP.subtract)
        fr = wrk.tile([128, SJW], F32, tag=f"fr{tag}")
        nc.vector.tensor_tensor(fr[:], v[:], vf[:], op=OP.subtract)
        return vf, fr

    def super_tile(g, s4):
        """Weights + wrapped idx for 512 output cols of row-group g.
        Returns (wxp, wy, C_idx) tiles."""
        IG = g * 128
        jb4 = s4 * SJW
        gt = gridp.tile([128, SJW * 2], F32, tag="gt")
        eng().dma_start(
            gt[:],
            bass.AP(gr, IG * W * 2 + jb4 * 2, [[W * 2, 128], [1, SJW * 2]]))
        gx = bass.AP(gt.tensor, gt.offset, [gt.ap[0], [2, SJW]])
        gy = bass.AP(gt.tensor, gt.offset + 1, [gt.ap[0], [2, SJW]])

        lx = wrk.tile([128, SJW], F32, tag="lx")
        ly = wrk.tile([128, SJW], F32, tag="ly")
        nc.scalar.activation(lx[:], gx, Copy, bias=1047.5 - IG, scale=1024.0)
        nc.scalar.activation(ly[:], gy, Copy, bias=1046.5 - jb4, scale=1024.0)
        fx, tx = floorpair(lx, "x")
        fy, ty = floorpair(ly, "y")

        # idxf = fy*XT + fx - 1 (f32, exact)
        idxf = wrk.tile([128, SJW], F32, tag="idxf")
        nc.vector.scalar_tensor_tensor(idxf[:], fy[:], float(XT), fx[:],
                                       op0=OP.mult, op1=OP.add)
        # per-sub-tile rebase to the 186-row gather window, cast to i16
        idx16 = idxs1.tile([128, SJW], I16, tag="idx16")
        for t in range(SJW // 128):
            nc.vector.tensor_scalar(
                bass.AP(idx16.tensor, idx16.offset + t * 128,
                        [[idx16.ap[0][0], 128], [1, 128]]),
                bass.AP(idxf.tensor, idxf.offset + t * 128,
                        [[idxf.ap[0][0], 128], [1, 128]]),
                -1.0 - t * 128.0 * XT, None, op0=OP.add)

        # fold [128, SJW] -> wrapped [16, 8*SJW]: D[p, k*SJW + j] = idx16[16k+p, j]
        D = idxs1.tile([128, 8 * SJW], I16, tag="D")
        for k in range(8):
            src = bass.AP(idx16.tensor,
                          idx16.offset + 16 * k * idx16.ap[0][0],
                          [[idx16.ap[0][0], 16], [1, SJW]])
            dst = bass.AP(D.tensor, D.offset + k * SJW,
                          [[D.ap[0][0], 16], [1, SJW]])
            eng().dma_start(dst, src)
        # interleave: Cw[p, 8j+k] = D[p, k*SJW + j]  (one strided copy)
        Cw = idxp.tile([128, 8 * SJW], I16, tag="Cw")
        nc.vector.tensor_copy(
            bass.AP(Cw.tensor, Cw.offset, [[Cw.ap[0][0], 16], [8, SJW], [1, 8]]),
            bass.AP(D.tensor, D.offset, [[D.ap[0][0], 16], [1, SJW], [SJW, 8]]))
        # replicate to all 8 gpsimd cores
        for rep in range(1, 8):
            src = bass.AP(Cw.tensor, Cw.offset, [[Cw.ap[0][0], 16], [1, 8 * SJW]])
            dst = bass.AP(Cw.tensor, Cw.offset + 16 * rep * Cw.ap[0][0],
                          [[Cw.ap[0][0], 16], [1, 8 * SJW]])
            eng().dma_start(dst, src)

        wx = cubic(tx, "x", F32)
        wy = cubic(ty, "y", F16)
        # wxp[j, s] packed s-minor, f16
        wxp = wpp.tile([128, SJW * 4], F16, tag="wxp")
        for s in range(4):
            dst = bass.AP(wxp.tensor, wxp.offset + s, [wxp.ap[0], [4, SJW]])
            nc.scalar.copy(dst, wx[s][:])
        return wxp, wy, Cw

    def half_tile(g, s4, t, h, wxp, wy, Cw):
        IG = g * 128
        jb = s4 * SJW + t * 128 + h * JW2
        ybase = s4 * SJW + t * 128
        # wp_h[j, s, r] = wxp[j, s] * wy_r[j]  (JW2 cols)
        joff = t * 128 + h * JW2
        wp = wpp.tile([128, JW2 * 16], F16, tag="wp")
        for r in range(4):
            dst = bass.AP(wp.tensor, wp.offset + r,
                          [wp.ap[0], [16, JW2], [4, 4]])
            src0 = bass.AP(wxp.tensor, wxp.offset + joff * 4,
                           [wxp.ap[0], [4, JW2], [1, 4]])
            src1 = bass.AP(wy[r].tensor, wy[r].offset + joff,
                           [wy[r].ap[0], [1, JW2], [0, 4]])
            nc.vector.tensor_tensor(dst, src0, src1, op=OP.mult)

        NI = 128 * JW2  # 8192
        G = gp.tile([128, JW2 * 128], F16, tag="G")
        in_ap = bass.AP(tabs[g].tensor,
                        tabs[g].offset + ybase * XT * 128,
                        [[128, 186 * XT], [1, 128]])
        idxs = bass.AP(Cw.tensor, Cw.offset + (t * 128 + h * JW2) * 8,
                       [[Cw.ap[0][0], 128], [1, NI // 16]])
        q = cnt["q"] % 4
        cnt["q"] += 1
        nc.gpsimd.dma_gather(
            out_ap=bass.AP(G.tensor, G.offset,
                           [[G.ap[0][0], 128], [128, JW2], [1, 128]]),
            in_ap=in_ap,
            idxs_ap=idxs,
            num_idxs=NI,
            num_idxs_reg=NI,
            elem_size=128,
            elem_step=128,
            single_packet=False,
            queue_num=q,
        )

        # combine: P = G * wp (bcast over c), tree-reduce s then r
        src1 = bass.AP(wp.tensor, wp.offset,
                       [wp.ap[0], [16, JW2], [4, 4], [1, 4], [0, 8]])
        src0 = bass.AP(G.tensor, G.offset,
                       [G.ap[0], [128, JW2], [32, 4], [8, 4], [1, 8]])
        nc.vector.tensor_tensor(src0, src0, src1, op=OP.mult)

        def halve(buf, stride, n, tag, npx=JW2):
            o = lp.tile([128, npx * stride * (n // 2)], F16, tag=tag)
            i0 = bass.AP(buf.tensor, buf.offset,
                         [buf.ap[0], [stride * n, npx], [stride * 2, n // 2], [1, stride]])
            i1 = bass.AP(buf.tensor, buf.offset + stride,
                         [buf.ap[0], [stride * n, npx], [stride * 2, n // 2], [1, stride]])
            od = bass.AP(o.tensor, o.offset,
                         [o.ap[0], [stride * (n // 2), npx], [stride, n // 2], [1, stride]])
            nc.vector.tensor_tensor(od, i0, i1, op=OP.add)
            return o

        L1 = halve(G, 32, 4, "L1")
        L2 = halve(L1, 32, 2, "L2")
        L3 = halve(L2, 8, 4, "L3")
        of = outp.tile([128, 8 * JW2], F32, tag="of")
        i0 = bass.AP(L3.tensor, L3.offset, [L3.ap[0], [16, JW2], [1, 8]])
        i1 = bass.AP(L3.tensor, L3.offset + 8, [L3.ap[0], [16, JW2], [1, 8]])
        od = bass.AP(of.tensor, of.offset, [of.ap[0], [1, JW2], [JW2, 8]])
        nc.vector.tensor_tensor(od, i0, i1, op=OP.add)

        dsto = bass.AP(out, IG * W + jb,
                       [[W, 128], [RPC * W, 8], [1, JW2]])
        eng().dma_start(dsto, of[:])

    # ---------------- emission schedule --------------------------------
    for yb in range(N_YB):
        build_block(0, yb)

    # run g0 interleaved with build of g1
    runs = [(s4, t, h) for s4 in range(4) for t in range(4) for h in range(2)]
    built = 0
    sup = None
    for i, (s4, t, h) in enumerate(runs):
        if t == 0 and h == 0:
            sup = super_tile(0, s4)
        half_tile(0, s4, t, h, *sup)
        want = (i + 1) * N_YB // len(runs)
        while built < want:
            build_block(1, built)
            built += 1
    while built < N_YB:
        build_block(1, built)
        built += 1
    for s4 in range(4):
        sup = super_tile(1, s4)
        for t in range(4):
            for h in range(2):
                half_tile(1, s4, t, h, *sup)


_NC_CACHE = None


def kernel(x: np.ndarray, grid: np.ndarray) -> np.ndarray:
    global _NC_CACHE
    if _NC_CACHE is None:
        _NC_CACHE = build_nc()
    nc = _NC_CACHE

    x0 = np.ascontiguousarray(x[0], dtype=np.float32)        # [C, H, W]
    g0 = np.ascontiguousarray(grid[0], dtype=np.float32)     # [H, W, 2]

    in_maps = []
    for k in range(N_CORES):
        I0 = k * RPC
        xsl = np.zeros((C, YS + 4, XS), dtype=np.float32)
        c0 = I0 - PAD
        lo, hi = max(0, c0), min(W, c0 + XS)
        xsl[:, PAD:PAD + H, lo - c0:hi - c0] = x0[:, :, lo:hi]
        grc = np.ascontiguousarray(g0[I0:I0 + RPC]).copy()
        grc[..., 0] -= I0 / 1024.0   # fold per-core x-base into gx
        in_maps.append({"xs": xsl, "gr": grc})

    res = run_bass_kernel_spmd(nc, in_maps, core_ids=list(range(N_CORES)),
                               trace=False)
    global _LAST_EXEC_NS
    _LAST_EXEC_NS = res.exec_time_ns
    out = np.empty((1, C, H, W), dtype=np.float32)
    for k in range(N_CORES):
        out[0, :, k * RPC:(k + 1) * RPC, :] = res.results[k]["out"]
    return out


# revision 10
# speedup vs baseline: 1.1775x; 1.1775x over previous
"""Bicubic grid_sample (transpose-like warp) for Trainium2, 8 NeuronCores.

Strategy: shard output rows across cores (256 rows/core). The warp maps
output (i, j) -> input (y ~ j +- 21, x ~ i +- 21), so each core needs an
x-column slab of the image. On device, repack the slab into a patch table
in DRAM where each 256B unit holds the full 4x4x8ch bicubic patch at
(y0, x0) (fp16). v2: the table is built from a fully s,r-materialized
SBUF staging buffer and written via SWDGE with ~22KB contiguous
descriptors (16-engine spread); indices are folded into the gather's
wrapped 16-partition layout on-chip (no DRAM bounce); weights are
computed on 512-wide super-tiles; row-group 1's table build is emitted
interleaved with row-group 0's gather/combine so they overlap.
"""
import os, sys, types
sys.path.insert(0, "/opt/trn_rl_repo")
import numpy as np

try:  # register NTFF profile hook so BASS_TRACE=1 can measure HW time
    import antenv
    if "antenv.axon_hooks" not in sys.modules:
        from trn_agent_boot.trn_boot import _ntff_profile_via_ctypes
        _h = _ntff_profile_via_ctypes("/opt/axon/libaxon_pjrt.so")
        _m = types.ModuleType("antenv.axon_hooks")
        _m.get_axon_ntff_profile_hook = lambda: _h
        _m.set_axon_ntff_profile_hook = lambda h: None
        sys.modules["antenv.axon_hooks"] = _m
        antenv.axon_hooks = _m
except Exception:
    pass

import concourse.bass as bass
import concourse.bacc as bacc
import concourse.mybir as mybir
import concourse.tile as tile
from concourse import library_config
from concourse.bass_utils import run_bass_kernel_spmd

F32 = mybir.dt.float32
F16 = mybir.dt.float16
I16 = mybir.dt.int16
I32 = mybir.dt.int32
OP = mybir.AluOpType
ACTF = None  # set after import

N_CORES = 8
H = W = 2048
C = 8
RPC = H // N_CORES          # output rows per core = 256
PAD = 24                    # y halo rows on each side
YS = H + 2 * PAD            # 2096 slab rows
XS = 308                    # slab cols: [I0-24, I0+284)
XT = 176                    # table cols per row-group
XH = 88                     # x-half of the table staging buffer
YT = YS + 16                # table rows incl. pad so in_ap window stays in-bounds
SJW = 512                   # super-tile width (weights/idx granularity)
JW2 = 64                    # half-tile width (gather/combine granularity)
A = -0.75                   # bicubic constant
YB = 124                    # y-block rows for table build
N_YB = (YS + YB - 1) // YB  # 17


def build_nc():
    nc = bacc.Bacc("TRN2", target_bir_lowering=False, debug=False,
                   num_devices=N_CORES, num_swdge_queues=4)
    xs = nc.dram_tensor("xs", [C, YS + 4, XS], F32, kind="ExternalInput")
    gr = nc.dram_tensor("gr", [RPC, W, 2], F32, kind="ExternalInput")
    out = nc.dram_tensor("out", [C, RPC, W], F32, kind="ExternalOutput")

    with tile.TileContext(nc) as tc:
        nc.gpsimd.load_library(library_config.mlp)
        import contextlib
        with contextlib.ExitStack() as ctx:
            _build_body(ctx, tc, nc, xs, gr, out)
    nc.compile()
    return nc


def _build_body(ctx, tc, nc, xs, gr, out):
    Copy = mybir.ActivationFunctionType.Copy
    tabpool = ctx.enter_context(tc.tile_pool(name="tab", bufs=1, space="DRAM"))
    # phase-1 pools
    tpool = ctx.enter_context(tc.tile_pool(name="t", bufs=1))
    tgpool = ctx.enter_context(tc.tile_pool(name="tg", bufs=1))
    tg2pool = ctx.enter_context(tc.tile_pool(name="tg2", bufs=1))
    # phase-2 pools
    gridp = ctx.enter_context(tc.tile_pool(name="grid", bufs=2))
    wrk = ctx.enter_context(tc.tile_pool(name="wrk", bufs=1))
    wpp = ctx.enter_context(tc.tile_pool(name="wpp", bufs=2))
    idxp = ctx.enter_context(tc.tile_pool(name="idx", bufs=2))
    idxs1 = ctx.enter_context(tc.tile_pool(name="idx1", bufs=1))
    gp = ctx.enter_context(tc.tile_pool(name="g", bufs=2))
    lp = ctx.enter_context(tc.tile_pool(name="l", bufs=1))
    outp = ctx.enter_context(tc.tile_pool(name="out", bufs=2))

    tabs = []
    for g in range(2):
        tabg = tabpool.tile([YT * XT, 128], F16, tag=f"tab{g}")
        tabs.append(tabg)

    hwdge = [nc.sync, nc.scalar]
    cnt = {"dma": 0, "cp": 0, "q": 0}

    def eng():
        cnt["dma"] += 1
        return hwdge[cnt["dma"] % 2]

    def ccopy(dst, src):
        cnt["cp"] += 1
        if cnt["cp"] % 2 == 0:
            nc.vector.tensor_copy(dst, src)
        else:
            nc.scalar.copy(dst, src)

    # ---------------- phase 1: repack xs -> table[g], one y-block ----------
    def build_block(g, yb):
        y0 = yb * YB
        rows = min(YB, YS - y0)
        tg = tgpool.tile([128, 179 * 32], F16, tag="tg")
        for r in range(4):
            # one DMA for all 8 channels of row-shift r: t[p, c*179 + x]
            t = tpool.tile([128, 8 * 179], F32, tag=f"xsb{r % 2}")
            eng().dma_start(
                bass.AP(t.tensor, t.offset, [[t.ap[0][0], rows], [1, 8 * 179]]),
                bass.AP(xs, (y0 + r) * XS + 128 * g,
                        [[XS, rows], [(YS + 4) * XS, 8], [1, 179]]))
            # one interleave+cast op: tg[p, xu*32 + r*8 + c] = t[p, c*179+xu]
            dst = bass.AP(tg.tensor, tg.offset + r * 8,
                          [[tg.ap[0][0], rows], [1, 8], [32, 179]])
            srcap = bass.AP(t.tensor, t.offset,
                            [[t.ap[0][0], rows], [179, 8], [1, 179]])
            ccopy(dst, srcap)
        for h in range(2):
            tg2 = tg2pool.tile([128, XH * 128], F16, tag=f"tg2{h}")
            for s in range(4):
                src = bass.AP(tg.tensor, tg.offset + (XH * h + s) * 32,
                              [[tg.ap[0][0], rows], [32, XH], [1, 32]])
                dst = bass.AP(tg2.tensor, tg2.offset + s * 32,
                              [[tg2.ap[0][0], rows], [128, XH], [1, 32]])
                ccopy(dst, src)
            # one HWDGE DMA, contiguous 22.5KB per row on both sides
            dsta = bass.AP(tabs[g].tensor,
                           tabs[g].offset + (y0 * XT + h * XH) * 128,
                           [[XT * 128, rows], [1, XH * 128]])
            eng().dma_start(dsta, tg2[:rows, :])

    # ---------------- phase 2: per super-tile weights+idx, gather+combine --
    def cubic(t, tag, outdt, opool):
        # returns w0..w3 tiles [128, SJW] in outdt; all-DVE (no cross-engine
        # handoffs); scratch tags shared between calls (sequential use).
        TS = nc.vector.tensor_scalar
        TT = nc.vector.tensor_tensor
        s0 = wrk.tile([128, SJW], F32, tag="c_s0")
        TS(s0[:], t[:], 1.0, None, op0=OP.add)
        w0f = wrk.tile([128, SJW], F32, tag="c_w0f")
        TS(w0f[:], s0[:], A, -5.0 * A, op0=OP.mult, op1=OP.add)
        TT(w0f[:], w0f[:], s0[:], op=OP.mult)
        TS(w0f[:], w0f[:], 8.0 * A, None, op0=OP.add)
        TT(w0f[:], w0f[:], s0[:], op=OP.mult)
        w0 = opool.tile([128, SJW], outdt, tag=f"w0{tag}")
        TS(w0[:], w0f[:], -4.0 * A, None, op0=OP.add)
        # w1
        w1f = wrk.tile([128, SJW], F32, tag="c_w1f")
        TS(w1f[:], t[:], A + 2.0, -(A + 3.0), op0=OP.mult, op1=OP.add)
        t2 = wrk.tile([128, SJW], F32, tag="c_t2")
        TT(t2[:], t[:], t[:], op=OP.mult)
        TT(w1f[:], w1f[:], t2[:], op=OP.mult)
        w1 = opool.tile([128, SJW], outdt, tag=f"w1{tag}")
        TS(w1[:], w1f[:], 1.0, None, op0=OP.add)
        # w2: u = 1 - t
        u = wrk.tile([128, SJW], F32, tag="c_u")
        TS(u[:], t[:], -1.0, 1.0, op0=OP.mult, op1=OP.add)
        w2f = wrk.tile([128, SJW], F32, tag="c_w2f")
        TS(w2f[:], u[:], A + 2.0, -(A + 3.0), op0=OP.mult, op1=OP.add)
        u2 = wrk.tile([128, SJW], F32, tag="c_u2")
        TT(u2[:], u[:], u[:], op=OP.mult)
        TT(w2f[:], w2f[:], u2[:], op=OP.mult)
        w2 = opool.tile([128, SJW], outdt, tag=f"w2{tag}")
        TS(w2[:], w2f[:], 1.0, None, op0=OP.add)
        # w3 = 1 - w0 - w1 - w2 (in f32 then cast)
        w3f = wrk.tile([128, SJW], F32, tag="c_w3f")
        TT(w3f[:], w0[:], w1[:], op=OP.add)
        TT(w3f[:], w3f[:], w2[:], op=OP.add)
        w3 = opool.tile([128, SJW], outdt, tag=f"w3{tag}")
        TS(w3[:], w3f[:], -1.0, 1.0, op0=OP.mult, op1=OP.add)
        return [w0, w1, w2, w3]

    def floorpair(v, tag):
        # vi/co scratch shared between calls; vf/fr persist per-dir
        vi = wrk.tile([128, SJW], I32, tag="f_vi")
        nc.vector.tensor_copy(vi[:], v[:])
        vf = wrk.tile([128, SJW], F32, tag=f"vf{tag}")
        nc.vector.tensor_copy(vf[:], vi[:])
        co = wrk.tile([128, SJW], F32, tag="f_co")
        nc.vector.tensor_tensor(co[:], vf[:], v[:], op=OP.is_gt)
        nc.vector.tensor_tensor(vf[:], vf[:], co[:], op=OP.subtract)
        fr = wrk.tile([128, SJW], F32, tag=f"fr{tag}")
        nc.vector.tensor_tensor(fr[:], v[:], vf[:], op=OP.subtract)
        return vf, fr

    def super_tile(g, s4):
        """Weights + wrapped idx for 512 output cols of row-group g.
        Returns (wxp, wy, C_idx) tiles."""
        IG = g * 128
        jb4 = s4 * SJW
        gt = gridp.tile([128, SJW * 2], F32, tag="gt")
        eng().dma_start(
            gt[:],
            bass.AP(gr, IG * W * 2 + jb4 * 2, [[W * 2, 128], [1, SJW * 2]]))
        gx = bass.AP(gt.tensor, gt.offset, [gt.ap[0], [2, SJW]])
        gy = bass.AP(gt.tensor, gt.offset + 1, [gt.ap[0], [2, SJW]])

        lx = wrk.tile([128, SJW], F32, tag="lx")
        ly = wrk.tile([128, SJW], F32, tag="ly")
        nc.vector.tensor_scalar(lx[:], gx, 1024.0, 1047.5 - IG,
                                op0=OP.mult, op1=OP.add)
        nc.vector.tensor_scalar(ly[:], gy, 1024.0, 1046.5 - jb4,
                                op0=OP.mult, op1=OP.add)
        fx, tx = floorpair(lx, "x")
        fy, ty = floorpair(ly, "y")

        # idxf = fy*XT + fx - 1 (f32, exact)
        idxf = wrk.tile([128, SJW], F32, tag="idxf")
        nc.vector.scalar_tensor_tensor(idxf[:], fy[:], float(XT), fx[:],
                                       op0=OP.mult, op1=OP.add)
        # per-sub-tile rebase to the 186-row gather window, cast to i16
        idx16 = idxs1.tile([128, SJW], I16, tag="idx16")
        for t in range(SJW // 128):
            nc.vector.tensor_scalar(
                bass.AP(idx16.tensor, idx16.offset + t * 128,
                        [[idx16.ap[0][0], 128], [1, 128]]),
                bass.AP(idxf.tensor, idxf.offset + t * 128,
                        [[idxf.ap[0][0], 128], [1, 128]]),
                -1.0 - t * 128.0 * XT, None, op0=OP.add)

        # fold [128, SJW] -> wrapped [16, 8*SJW]: D[p, k*SJW + j] = idx16[16k+p, j]
        D = idxs1.tile([128, 8 * SJW], I16, tag="D")
        for k in range(8):
            src = bass.AP(idx16.tensor,
                          idx16.offset + 16 * k * idx16.ap[0][0],
                          [[idx16.ap[0][0], 16], [1, SJW]])
            dst = bass.AP(D.tensor, D.offset + k * SJW,
                          [[D.ap[0][0], 16], [1, SJW]])
            eng().dma_start(dst, src)
        # interleave: Cw[p, 8j+k] = D[p, k*SJW + j]  (one strided copy)
        Cw = idxp.tile([128, 8 * SJW], I16, tag="Cw")
        nc.vector.tensor_copy(
            bass.AP(Cw.tensor, Cw.offset, [[Cw.ap[0][0], 16], [8, SJW], [1, 8]]),
            bass.AP(D.tensor, D.offset, [[D.ap[0][0], 16], [1, SJW], [SJW, 8]]))
        # replicate to all 8 gpsimd cores
        for rep in range(1, 8):
            src = bass.AP(Cw.tensor, Cw.offset, [[Cw.ap[0][0], 16], [1, 8 * SJW]])
            dst = bass.AP(Cw.tensor, Cw.offset + 16 * rep * Cw.ap[0][0],
                          [[Cw.ap[0][0], 16], [1, 8 * SJW]])
            eng().dma_start(dst, src)

        wx = cubic(tx, "x", F16, wrk)
        wy = cubic(ty, "y", F16, wpp)
        # wxp[j, s] packed s-minor, f16
        wxp = wpp.tile([128, SJW * 4], F16, tag="wxp")
        for s in range(4):
            dst = bass.AP(wxp.tensor, wxp.offset + s, [wxp.ap[0], [4, SJW]])
            nc.scalar.copy(dst, wx[s][:])
        return wxp, wy, Cw

    def half_tile(g, s4, t, h, wxp, wy, Cw):
        IG = g * 128
        jb = s4 * SJW + t * 128 + h * JW2
        ybase = s4 * SJW + t * 128
        # wp_h[j, s, r] = wxp[j, s] * wy_r[j]  (JW2 cols)
        joff = t * 128 + h * JW2
        wp = wpp.tile([128, JW2 * 16], F16, tag="wp")
        for r in range(4):
            dst = bass.AP(wp.tensor, wp.offset + r,
                          [wp.ap[0], [16, JW2], [4, 4]])
            src0 = bass.AP(wxp.tensor, wxp.offset + joff * 4,
                           [wxp.ap[0], [4, JW2], [1, 4]])
            src1 = bass.AP(wy[r].tensor, wy[r].offset + joff,
                           [wy[r].ap[0], [1, JW2], [0, 4]])
            nc.vector.tensor_tensor(dst, src0, src1, op=OP.mult)

        G = gp.tile([128, JW2 * 128], F16, tag="G")
        in_ap = bass.AP(tabs[g].tensor,
                        tabs[g].offset + ybase * XT * 128,
                        [[128, 186 * XT], [1, 128]])
        NSUB = 4096
        for m in range(2):
            idxs = bass.AP(Cw.tensor,
                           Cw.offset + (t * 128 + h * JW2) * 8 + m * (NSUB // 16),
                           [[Cw.ap[0][0], 128], [1, NSUB // 16]])
            q = cnt["q"] % 4
            cnt["q"] += 1
            nc.gpsimd.dma_gather(
                out_ap=bass.AP(G.tensor, G.offset + m * 32 * 128,
                               [[G.ap[0][0], 128], [128, 32], [1, 128]]),
                in_ap=in_ap,
                idxs_ap=idxs,
                num_idxs=NSUB,
                num_idxs_reg=NSUB,
                elem_size=128,
                elem_step=128,
                single_packet=False,
                queue_num=q,
            )

        # combine: P = G * wp (bcast over c), tree-reduce s then r
        src1 = bass.AP(wp.tensor, wp.offset,
                       [wp.ap[0], [16, JW2], [4, 4], [1, 4], [0, 8]])
        src0 = bass.AP(G.tensor, G.offset,
                       [G.ap[0], [128, JW2], [32, 4], [8, 4], [1, 8]])
        nc.vector.tensor_tensor(src0, src0, src1, op=OP.mult)

        def halve(buf, stride, n, tag, npx=JW2):
            o = lp.tile([128, npx * stride * (n // 2)], F16, tag=tag)
            i0 = bass.AP(buf.tensor, buf.offset,
                         [buf.ap[0], [stride * n, npx], [stride * 2, n // 2], [1, stride]])
            i1 = bass.AP(buf.tensor, buf.offset + stride,
                         [buf.ap[0], [stride * n, npx], [stride * 2, n // 2], [1, stride]])
            od = bass.AP(o.tensor, o.offset,
                         [o.ap[0], [stride * (n // 2), npx], [stride, n // 2], [1, stride]])
            nc.vector.tensor_tensor(od, i0, i1, op=OP.add)
            return o

        L1 = halve(G, 32, 4, "L1")
        L2 = halve(L1, 32, 2, "L2")
        L3 = halve(L2, 8, 4, "L3")
        of = outp.tile([128, 8 * JW2], F32, tag="of")
        i0 = bass.AP(L3.tensor, L3.offset, [L3.ap[0], [16, JW2], [1, 8]])
        i1 = bass.AP(L3.tensor, L3.offset + 8, [L3.ap[0], [16, JW2], [1, 8]])
        od = bass.AP(of.tensor, of.offset, [of.ap[0], [1, JW2], [JW2, 8]])
        nc.vector.tensor_tensor(od, i0, i1, op=OP.add)

        dsto = bass.AP(out, IG * W + jb,
                       [[W, 128], [RPC * W, 8], [1, JW2]])
        eng().dma_start(dsto, of[:])

    # ---------------- emission schedule --------------------------------
    for yb in range(N_YB):
        build_block(0, yb)

    # run g0 interleaved with build of g1
    runs = [(s4, t, h) for s4 in range(4) for t in range(4) for h in range(2)]
    built = 0
    sup = None
    for i, (s4, t, h) in enumerate(runs):
        if t == 0 and h == 0:
            sup = super_tile(0, s4)
        half_tile(0, s4, t, h, *sup)
        want = (i + 1) * N_YB // len(runs)
        while built < want:
            build_block(1, built)
            built += 1
    while built < N_YB:
        build_block(1, built)
        built += 1
    for s4 in range(4):
        sup = super_tile(1, s4)
        for t in range(4):
            for h in range(2):
                half_tile(1, s4, t, h, *sup)


_NC_CACHE = None


def kernel(x: np.ndarray, grid: np.ndarray) -> np.ndarray:
    global _NC_CACHE
    if _NC_CACHE is None:
        _NC_CACHE = build_nc()
    nc = _NC_CACHE

    x0 = np.ascontiguousarray(x[0], dtype=np.float32)        # [C, H, W]
    g0 = np.ascontiguousarray(grid[0], dtype=np.float32)     # [H, W, 2]

    in_maps = []
    for k in range(N_CORES):
        I0 = k * RPC
        xsl = np.zeros((C, YS + 4, XS), dtype=np.float32)
        c0 = I0 - PAD
        lo, hi = max(0, c0), min(W, c0 + XS)
        xsl[:, PAD:PAD + H, lo - c0:hi - c0] = x0[:, :, lo:hi]
        grc = np.ascontiguousarray(g0[I0:I0 + RPC]).copy()
        grc[..., 0] -= I0 / 1024.0   # fold per-core x-base into gx
        in_maps.append({"xs": xsl, "gr": grc})

    res = run_bass_kernel_spmd(nc, in_maps, core_ids=list(range(N_CORES)),
                               trace=False)
    global _LAST_EXEC_NS
    _LAST_EXEC_NS = res.exec_time_ns
    out = np.empty((1, C, H, W), dtype=np.float32)
    for k in range(N_CORES):
        out[0, :, k * RPC:(k + 1) * RPC, :] = res.results[k]["out"]
    return out


# revision 13
# speedup vs baseline: 1.2123x; 1.0295x over previous
"""Bicubic grid_sample (transpose-like warp) for Trainium2, 8 NeuronCores.

Strategy: shard output rows across cores (256 rows/core). The warp maps
output (i, j) -> input (y ~ j +- 21, x ~ i +- 21), so each core needs an
x-column slab of the image. On device, repack the slab into a patch table
in DRAM where each 256B unit holds the full 4x4x8ch bicubic patch at
(y0, x0) (fp16). v2: the table is built from a fully s,r-materialized
SBUF staging buffer and written via SWDGE with ~22KB contiguous
descriptors (16-engine spread); indices are folded into the gather's
wrapped 16-partition layout on-chip (no DRAM bounce); weights are
computed on 512-wide super-tiles; row-group 1's table build is emitted
interleaved with row-group 0's gather/combine so they overlap.
"""
import os, sys, types
sys.path.insert(0, "/opt/trn_rl_repo")
import numpy as np

try:  # register NTFF profile hook so BASS_TRACE=1 can measure HW time
    import antenv
    if "antenv.axon_hooks" not in sys.modules:
        from trn_agent_boot.trn_boot import _ntff_profile_via_ctypes
        _h = _ntff_profile_via_ctypes("/opt/axon/libaxon_pjrt.so")
        _m = types.ModuleType("antenv.axon_hooks")
        _m.get_axon_ntff_profile_hook = lambda: _h
        _m.set_axon_ntff_profile_hook = lambda h: None
        sys.modules["antenv.axon_hooks"] = _m
        antenv.axon_hooks = _m
except Exception:
    pass

import concourse.bass as bass
import concourse.bacc as bacc
import concourse.mybir as mybir
import concourse.tile as tile
from concourse import library_config
from concourse.bass_utils import run_bass_kernel_spmd

F32 = mybir.dt.float32
F16 = mybir.dt.float16
I16 = mybir.dt.int16
I32 = mybir.dt.int32
OP = mybir.AluOpType
ACTF = None  # set after import

N_CORES = 8
H = W = 2048
C = 8
RPC = H // N_CORES          # output rows per core = 256
PAD = 24                    # y halo rows on each side
YS = H + 2 * PAD            # 2096 slab rows
XS = 308                    # slab cols: [I0-24, I0+284)
XT = 176                    # table cols per row-group
XH = 88                     # x-half of the table staging buffer
YT = YS + 16                # table rows incl. pad so in_ap window stays in-bounds
SJW = 512                   # super-tile width (weights/idx granularity)
JW2 = 64                    # half-tile width (gather/combine granularity)
A = -0.75                   # bicubic constant
YB = 124                    # y-block rows for table build
N_YB = (YS + YB - 1) // YB  # 17


def build_nc():
    nc = bacc.Bacc("TRN2", target_bir_lowering=False, debug=False,
                   num_devices=N_CORES, num_swdge_queues=4)
    xs = nc.dram_tensor("xs", [C, YS + 4, XS], F32, kind="ExternalInput")
    gr = nc.dram_tensor("gr", [RPC, W, 2], F32, kind="ExternalInput")
    out = nc.dram_tensor("out", [RPC, W // JW2, C, JW2], F32,
                         kind="ExternalOutput")

    with tile.TileContext(nc) as tc:
        nc.gpsimd.load_library(library_config.mlp)
        import contextlib
        with contextlib.ExitStack() as ctx:
            _build_body(ctx, tc, nc, xs, gr, out)
    nc.compile()
    return nc


def _build_body(ctx, tc, nc, xs, gr, out):
    Copy = mybir.ActivationFunctionType.Copy
    tabpool = ctx.enter_context(tc.tile_pool(name="tab", bufs=1, space="DRAM"))
    # phase-1 pools
    tpool = ctx.enter_context(tc.tile_pool(name="t", bufs=1))
    tg2pool = ctx.enter_context(tc.tile_pool(name="tg2", bufs=1))
    # phase-2 pools
    gridp = ctx.enter_context(tc.tile_pool(name="grid", bufs=2))
    wrk = ctx.enter_context(tc.tile_pool(name="wrk", bufs=1))
    wpp = ctx.enter_context(tc.tile_pool(name="wpp", bufs=2))
    idxp = ctx.enter_context(tc.tile_pool(name="idx", bufs=2))
    idxs1 = ctx.enter_context(tc.tile_pool(name="idx1", bufs=1))
    gp = ctx.enter_context(tc.tile_pool(name="g", bufs=2))
    lp = ctx.enter_context(tc.tile_pool(name="l", bufs=1))
    outp = ctx.enter_context(tc.tile_pool(name="out", bufs=2))

    tabs = []
    for g in range(2):
        tabg = tabpool.tile([YT * XT, 128], F16, tag=f"tab{g}")
        tabs.append(tabg)

    hwdge = [nc.sync, nc.scalar]
    cnt = {"dma": 0, "cp": 0, "q": 0}

    def eng():
        cnt["dma"] += 1
        return hwdge[cnt["dma"] % 2]

    def ccopy(dst, src):
        cnt["cp"] += 1
        if cnt["cp"] % 2 == 0:
            nc.vector.tensor_copy(dst, src)
        else:
            nc.scalar.copy(dst, src)

    # ---------------- phase 1: repack xs -> table[g], one y-block ----------
    def build_block(g, yb):
        y0 = yb * YB
        rows = min(YB, YS - y0)
        ts = []
        for r in range(4):
            # one DMA for all 8 channels of row-shift r: t[p, c*179 + x]
            t = tpool.tile([128, 8 * 179], F32, tag=f"xsb{r}")
            eng().dma_start(
                bass.AP(t.tensor, t.offset, [[t.ap[0][0], rows], [1, 8 * 179]]),
                bass.AP(xs, (y0 + r) * XS + 128 * g,
                        [[XS, rows], [(YS + 4) * XS, 8], [1, 179]]))
            ts.append(t)
        for h in range(2):
            tg2 = tg2pool.tile([128, XH * 128], F16, tag=f"tg2{h}")
            for r in range(4):
                # merged interleave+shift+cast:
                # tg2[p, xu*128 + s*32 + r*8 + c] = t[p, c*179 + XH*h + xu + s]
                dst = bass.AP(tg2.tensor, tg2.offset + r * 8,
                              [[tg2.ap[0][0], rows], [1, 8], [32, 4], [128, XH]])
                srcap = bass.AP(ts[r].tensor, ts[r].offset + XH * h,
                                [[ts[r].ap[0][0], rows], [179, 8], [1, 4], [1, XH]])
                ccopy(dst, srcap)
            # one HWDGE DMA, contiguous 22.5KB per row on both sides
            dsta = bass.AP(tabs[g].tensor,
                           tabs[g].offset + (y0 * XT + h * XH) * 128,
                           [[XT * 128, rows], [1, XH * 128]])
            eng().dma_start(dsta, tg2[:rows, :])

    # ---------------- phase 2: per super-tile weights+idx, gather+combine --
    def cubic(t, tag, outdt, opool):
        # returns w0..w3 tiles [128, SJW] in outdt; all-DVE (no cross-engine
        # handoffs); scratch tags shared between calls (sequential use).
        TS = nc.vector.tensor_scalar
        TT = nc.vector.tensor_tensor
        s0 = wrk.tile([128, SJW], F32, tag="c_s0")
        TS(s0[:], t[:], 1.0, None, op0=OP.add)
        w0f = wrk.tile([128, SJW], F32, tag="c_w0f")
        TS(w0f[:], s0[:], A, -5.0 * A, op0=OP.mult, op1=OP.add)
        TT(w0f[:], w0f[:], s0[:], op=OP.mult)
        TS(w0f[:], w0f[:], 8.0 * A, None, op0=OP.add)
        TT(w0f[:], w0f[:], s0[:], op=OP.mult)
        w0 = opool.tile([128, SJW], outdt, tag=f"w0{tag}")
        TS(w0[:], w0f[:], -4.0 * A, None, op0=OP.add)
        # w1
        w1f = wrk.tile([128, SJW], F32, tag="c_w1f")
        TS(w1f[:], t[:], A + 2.0, -(A + 3.0), op0=OP.mult, op1=OP.add)
        TT(w1f[:], w1f[:], t[:], op=OP.mult)
        TT(w1f[:], w1f[:], t[:], op=OP.mult)
        w1 = opool.tile([128, SJW], outdt, tag=f"w1{tag}")
        TS(w1[:], w1f[:], 1.0, None, op0=OP.add)
        # w2: u = 1 - t
        u = wrk.tile([128, SJW], F32, tag="c_u")
        TS(u[:], t[:], -1.0, 1.0, op0=OP.mult, op1=OP.add)
        w2f = wrk.tile([128, SJW], F32, tag="c_w2f")
        TS(w2f[:], u[:], A + 2.0, -(A + 3.0), op0=OP.mult, op1=OP.add)
        TT(w2f[:], w2f[:], u[:], op=OP.mult)
        TT(w2f[:], w2f[:], u[:], op=OP.mult)
        w2 = opool.tile([128, SJW], outdt, tag=f"w2{tag}")
        TS(w2[:], w2f[:], 1.0, None, op0=OP.add)
        # w3 = 1 - w0 - w1 - w2 (in f32 then cast)
        w3f = wrk.tile([128, SJW], F32, tag="c_w3f")
        TT(w3f[:], w0[:], w1[:], op=OP.add)
        TT(w3f[:], w3f[:], w2[:], op=OP.add)
        w3 = opool.tile([128, SJW], outdt, tag=f"w3{tag}")
        TS(w3[:], w3f[:], -1.0, 1.0, op0=OP.mult, op1=OP.add)
        return [w0, w1, w2, w3]

    def floorpair(v, tag):
        # vi/co scratch shared between calls; vf/fr persist per-dir
        vi = wrk.tile([128, SJW], I32, tag="f_vi")
        nc.vector.tensor_copy(vi[:], v[:])
        vf = wrk.tile([128, SJW], F32, tag=f"vf{tag}")
        nc.vector.tensor_copy(vf[:], vi[:])
        co = wrk.tile([128, SJW], F32, tag="f_co")
        nc.vector.tensor_tensor(co[:], vf[:], v[:], op=OP.is_gt)
        nc.vector.tensor_tensor(vf[:], vf[:], co[:], op=OP.subtract)
        fr = wrk.tile([128, SJW], F32, tag=f"fr{tag}")
        nc.vector.tensor_tensor(fr[:], v[:], vf[:], op=OP.subtract)
        return vf, fr

    def super_tile(g, s4):
        """Weights + wrapped idx for 512 output cols of row-group g.
        Returns (wxp, wy, C_idx) tiles."""
        IG = g * 128
        jb4 = s4 * SJW
        gt = gridp.tile([128, SJW * 2], F32, tag="gt")
        eng().dma_start(
            gt[:],
            bass.AP(gr, IG * W * 2 + jb4 * 2, [[W * 2, 128], [1, SJW * 2]]))
        gx = bass.AP(gt.tensor, gt.offset, [gt.ap[0], [2, SJW]])
        gy = bass.AP(gt.tensor, gt.offset + 1, [gt.ap[0], [2, SJW]])

        lx = wrk.tile([128, SJW], F32, tag="lx")
        ly = wrk.tile([128, SJW], F32, tag="ly")
        nc.vector.tensor_scalar(lx[:], gx, 1024.0, 1047.5 - IG,
                                op0=OP.mult, op1=OP.add)
        nc.vector.tensor_scalar(ly[:], gy, 1024.0, 1046.5 - jb4,
                                op0=OP.mult, op1=OP.add)
        fx, tx = floorpair(lx, "x")
        fy, ty = floorpair(ly, "y")

        # idxf = fy*XT + fx - 1 (f32, exact)
        idxf = wrk.tile([128, SJW], F32, tag="idxf")
        nc.vector.scalar_tensor_tensor(idxf[:], fy[:], float(XT), fx[:],
                                       op0=OP.mult, op1=OP.add)
        # per-sub-tile rebase to the 186-row gather window, cast to i16
        idx16 = idxs1.tile([128, SJW], I16, tag="idx16")
        for t in range(SJW // 128):
            nc.vector.tensor_scalar(
                bass.AP(idx16.tensor, idx16.offset + t * 128,
                        [[idx16.ap[0][0], 128], [1, 128]]),
                bass.AP(idxf.tensor, idxf.offset + t * 128,
                        [[idxf.ap[0][0], 128], [1, 128]]),
                -1.0 - t * 128.0 * XT, None, op0=OP.add)

        # fold [128, SJW] -> wrapped [16, 8*SJW]: D[p, k*SJW + j] = idx16[16k+p, j]
        D = idxs1.tile([128, 8 * SJW], I16, tag="D")
        for k in range(8):
            src = bass.AP(idx16.tensor,
                          idx16.offset + 16 * k * idx16.ap[0][0],
                          [[idx16.ap[0][0], 16], [1, SJW]])
            dst = bass.AP(D.tensor, D.offset + k * SJW,
                          [[D.ap[0][0], 16], [1, SJW]])
            eng().dma_start(dst, src)
        # interleave: Cw[p, 8j+k] = D[p, k*SJW + j]  (one strided copy)
        Cw = idxp.tile([128, 8 * SJW], I16, tag="Cw")
        nc.vector.tensor_copy(
            bass.AP(Cw.tensor, Cw.offset, [[Cw.ap[0][0], 16], [8, SJW], [1, 8]]),
            bass.AP(D.tensor, D.offset, [[D.ap[0][0], 16], [1, SJW], [SJW, 8]]))
        # replicate to all 8 gpsimd cores
        for rep in range(1, 8):
            src = bass.AP(Cw.tensor, Cw.offset, [[Cw.ap[0][0], 16], [1, 8 * SJW]])
            dst = bass.AP(Cw.tensor, Cw.offset + 16 * rep * Cw.ap[0][0],
                          [[Cw.ap[0][0], 16], [1, 8 * SJW]])
            eng().dma_start(dst, src)

        wx = cubic(tx, "x", F16, wrk)
        wy = cubic(ty, "y", F16, wpp)
        # wxp[j, s] packed s-minor, f16
        wxp = wpp.tile([128, SJW * 4], F16, tag="wxp")
        for s in range(4):
            dst = bass.AP(wxp.tensor, wxp.offset + s, [wxp.ap[0], [4, SJW]])
            nc.scalar.copy(dst, wx[s][:])
        return wxp, wy, Cw

    def half_tile(g, s4, t, h, wxp, wy, Cw):
        IG = g * 128
        jb = s4 * SJW + t * 128 + h * JW2
        ybase = s4 * SJW + t * 128
        # wp_h[j, s, r] = wxp[j, s] * wy_r[j]  (JW2 cols)
        joff = t * 128 + h * JW2
        wp = wpp.tile([128, JW2 * 16], F16, tag="wp")
        for r in range(4):
            dst = bass.AP(wp.tensor, wp.offset + r,
                          [wp.ap[0], [16, JW2], [4, 4]])
            src0 = bass.AP(wxp.tensor, wxp.offset + joff * 4,
                           [wxp.ap[0], [4, JW2], [1, 4]])
            src1 = bass.AP(wy[r].tensor, wy[r].offset + joff,
                           [wy[r].ap[0], [1, JW2], [0, 4]])
            nc.vector.tensor_tensor(dst, src0, src1, op=OP.mult)

        G = gp.tile([128, JW2 * 128], F16, tag="G")
        in_ap = bass.AP(tabs[g].tensor,
                        tabs[g].offset + ybase * XT * 128,
                        [[128, 186 * XT], [1, 128]])
        NSUB = 4096
        for m in range(2):
            idxs = bass.AP(Cw.tensor,
                           Cw.offset + (t * 128 + h * JW2) * 8 + m * (NSUB // 16),
                           [[Cw.ap[0][0], 128], [1, NSUB // 16]])
            q = cnt["q"] % 4
            cnt["q"] += 1
            nc.gpsimd.dma_gather(
                out_ap=bass.AP(G.tensor, G.offset + m * 32 * 128,
                               [[G.ap[0][0], 128], [128, 32], [1, 128]]),
                in_ap=in_ap,
                idxs_ap=idxs,
                num_idxs=NSUB,
                num_idxs_reg=NSUB,
                elem_size=128,
                elem_step=128,
                single_packet=False,
                queue_num=q,
            )

        # combine: P = G * wp (bcast over c), tree-reduce s then r
        src1 = bass.AP(wp.tensor, wp.offset,
                       [wp.ap[0], [16, JW2], [4, 4], [1, 4], [0, 8]])
        src0 = bass.AP(G.tensor, G.offset,
                       [G.ap[0], [128, JW2], [32, 4], [8, 4], [1, 8]])
        nc.vector.tensor_tensor(src0, src0, src1, op=OP.mult)

        def halve(buf, stride, n, tag, npx=JW2):
            o = lp.tile([128, npx * stride * (n // 2)], F16, tag=tag)
            i0 = bass.AP(buf.tensor, buf.offset,
                         [buf.ap[0], [stride * n, npx], [stride * 2, n // 2], [1, stride]])
            i1 = bass.AP(buf.tensor, buf.offset + stride,
                         [buf.ap[0], [stride * n, npx], [stride * 2, n // 2], [1, stride]])
            od = bass.AP(o.tensor, o.offset,
                         [o.ap[0], [stride * (n // 2), npx], [stride, n // 2], [1, stride]])
            nc.vector.tensor_tensor(od, i0, i1, op=OP.add)
            return o

        L1 = halve(G, 32, 4, "L1")
        L2 = halve(L1, 32, 2, "L2")
        L3 = halve(L2, 8, 4, "L3")
        of = outp.tile([128, 8 * JW2], F32, tag="of")
        i0 = bass.AP(L3.tensor, L3.offset, [L3.ap[0], [16, JW2], [1, 8]])
        i1 = bass.AP(L3.tensor, L3.offset + 8, [L3.ap[0], [16, JW2], [1, 8]])
        od = bass.AP(of.tensor, of.offset, [of.ap[0], [1, JW2], [JW2, 8]])
        nc.vector.tensor_tensor(od, i0, i1, op=OP.add)

        # blocked layout [RPC, W//64, C, 64]: 2KB contiguous per partition
        dsto = bass.AP(out, (IG * (W // JW2) + jb // JW2) * C * JW2,
                       [[(W // JW2) * C * JW2, 128], [1, C * JW2]])
        eng().dma_start(dsto, of[:])

    # ---------------- emission schedule --------------------------------
    # Lockstep: table blocks are emitted just-in-time for the j-window each
    # run-super needs (y-prefix dependency); each super's weights/idx are
    # emitted one run-super ahead so they compute during the previous
    # super's gathers/combines.
    run_order = [(0, 0), (1, 0), (0, 1), (1, 1), (0, 2), (1, 2), (0, 3), (1, 3)]
    # blocks of table g needed before running super s4 (window jb4+512+150+pad)
    BLK_NEED = [min(N_YB, (512 * (s + 1) + 150 + PAD + YB - 1) // YB + 1)
                for s in range(4)]
    built = {0: 0, 1: 0}

    def ensure(g, upto):
        while built[g] < upto:
            build_block(g, built[g])
            built[g] += 1

    sups = {}
    ensure(0, BLK_NEED[0])
    sups[(0, 0)] = super_tile(0, 0)
    ensure(1, BLK_NEED[0])
    sups[(1, 0)] = super_tile(1, 0)
    for i, (g, s4) in enumerate(run_order):
        # emit next run-super's weights now (computes during our gathers)
        if i + 1 < len(run_order):
            gn, sn = run_order[i + 1]
            if (gn, sn) not in sups:
                ensure(gn, BLK_NEED[sn])
                sups[(gn, sn)] = super_tile(gn, sn)
        # drip remaining builds of the *other* table between half-tiles
        halves = [(t, h) for t in range(4) for h in range(2)]
        og = 1 - g
        need_more = (N_YB - built[og]) if i >= len(run_order) - 3 else 0
        for k, (t, h) in enumerate(halves):
            half_tile(g, s4, t, h, *sups[(g, s4)])
            if need_more and k % 3 == 2 and built[og] < N_YB:
                build_block(og, built[og])
                built[og] += 1
        del sups[(g, s4)]



_NC_CACHE = None


def kernel(x: np.ndarray, grid: np.ndarray) -> np.ndarray:
    global _NC_CACHE
    if _NC_CACHE is None:
        _NC_CACHE = build_nc()
    nc = _NC_CACHE

    x0 = np.ascontiguousarray(x[0], dtype=np.float32)        # [C, H, W]
    g0 = np.ascontiguousarray(grid[0], dtype=np.float32)     # [H, W, 2]

    in_maps = []
    for k in range(N_CORES):
        I0 = k * RPC
        xsl = np.zeros((C, YS + 4, XS), dtype=np.float32)
        c0 = I0 - PAD
        lo, hi = max(0, c0), min(W, c0 + XS)
        xsl[:, PAD:PAD + H, lo - c0:hi - c0] = x0[:, :, lo:hi]
        grc = np.ascontiguousarray(g0[I0:I0 + RPC]).copy()
        grc[..., 0] -= I0 / 1024.0   # fold per-core x-base into gx
        in_maps.append({"xs": xsl, "gr": grc})

    res = run_bass_kernel_spmd(nc, in_maps, core_ids=list(range(N_CORES)),
                               trace=False)
    global _LAST_EXEC_NS
    _LAST_EXEC_NS = res.exec_time_ns
    out = np.empty((1, C, H, W), dtype=np.float32)
    for k in range(N_CORES):
        blk = res.results[k]["out"]          # [RPC, W//64, C, 64]
        out[0, :, k * RPC:(k + 1) * RPC, :] = (
            blk.transpose(2, 0, 1, 3).reshape(C, RPC, W))
    return out


# revision 14
# speedup vs baseline: 1.2769x; 1.0533x over previous
"""Bicubic grid_sample (transpose-like warp) for Trainium2, 8 NeuronCores.

Strategy: shard output rows across cores (256 rows/core). The warp maps
output (i, j) -> input (y ~ j +- 21, x ~ i +- 21), so each core needs an
x-column slab of the image. On device, repack the slab into a patch table
in DRAM where each 256B unit holds the full 4x4x8ch bicubic patch at
(y0, x0) (fp16). v2: the table is built from a fully s,r-materialized
SBUF staging buffer and written via SWDGE with ~22KB contiguous
descriptors (16-engine spread); indices are folded into the gather's
wrapped 16-partition layout on-chip (no DRAM bounce); weights are
computed on 512-wide super-tiles; row-group 1's table build is emitted
interleaved with row-group 0's gather/combine so they overlap.
"""
import os, sys, types
sys.path.insert(0, "/opt/trn_rl_repo")
import numpy as np

try:  # register NTFF profile hook so BASS_TRACE=1 can measure HW time
    import antenv
    if "antenv.axon_hooks" not in sys.modules:
        from trn_agent_boot.trn_boot import _ntff_profile_via_ctypes
        _h = _ntff_profile_via_ctypes("/opt/axon/libaxon_pjrt.so")
        _m = types.ModuleType("antenv.axon_hooks")
        _m.get_axon_ntff_profile_hook = lambda: _h
        _m.set_axon_ntff_profile_hook = lambda h: None
        sys.modules["antenv.axon_hooks"] = _m
        antenv.axon_hooks = _m
except Exception:
    pass

import concourse.bass as bass
import concourse.bacc as bacc
import concourse.mybir as mybir
import concourse.tile as tile
from concourse import library_config
from concourse.bass_utils import run_bass_kernel_spmd

F32 = mybir.dt.float32
F16 = mybir.dt.float16
I16 = mybir.dt.int16
I32 = mybir.dt.int32
OP = mybir.AluOpType
ACTF = None  # set after import

N_CORES = 8
H = W = 2048
C = 8
RPC = H // N_CORES          # output rows per core = 256
PAD = 24                    # y halo rows on each side
YS = H + 2 * PAD            # 2096 slab rows
XS = 308                    # slab cols: [I0-24, I0+284)
XT = 176                    # table cols per row-group
XH = 88                     # x-half of the table staging buffer
YT = YS + 16                # table rows incl. pad so in_ap window stays in-bounds
SJW = 512                   # super-tile width (weights/idx granularity)
JW2 = 64                    # half-tile width (gather/combine granularity)
A = -0.75                   # bicubic constant
YB = 124                    # y-block rows for table build
N_YB = (YS + YB - 1) // YB  # 17


def build_nc():
    nc = bacc.Bacc("TRN2", target_bir_lowering=False, debug=False,
                   num_devices=N_CORES, num_swdge_queues=4)
    xs = nc.dram_tensor("xs", [C, YS + 4, XS], F32, kind="ExternalInput")
    gr = nc.dram_tensor("gr", [RPC, W, 2], F32, kind="ExternalInput")
    out = nc.dram_tensor("out", [RPC, W // JW2, C, JW2], F32,
                         kind="ExternalOutput")

    with tile.TileContext(nc) as tc:
        nc.gpsimd.load_library(library_config.mlp)
        import contextlib
        with contextlib.ExitStack() as ctx:
            _build_body(ctx, tc, nc, xs, gr, out)
    nc.compile()
    return nc


def _build_body(ctx, tc, nc, xs, gr, out):
    Copy = mybir.ActivationFunctionType.Copy
    tabpool = ctx.enter_context(tc.tile_pool(name="tab", bufs=1, space="DRAM"))
    # phase-1 pools
    tpool = ctx.enter_context(tc.tile_pool(name="t", bufs=1))
    tg2pool = ctx.enter_context(tc.tile_pool(name="tg2", bufs=1))
    # phase-2 pools
    gridp = ctx.enter_context(tc.tile_pool(name="grid", bufs=2))
    wrk = ctx.enter_context(tc.tile_pool(name="wrk", bufs=1))
    wpp = ctx.enter_context(tc.tile_pool(name="wpp", bufs=2))
    idxp = ctx.enter_context(tc.tile_pool(name="idx", bufs=2))
    idxs1 = ctx.enter_context(tc.tile_pool(name="idx1", bufs=1))
    gp = ctx.enter_context(tc.tile_pool(name="g", bufs=2))
    lp = ctx.enter_context(tc.tile_pool(name="l", bufs=1))
    outp = ctx.enter_context(tc.tile_pool(name="out", bufs=2))

    tabs = []
    for g in range(2):
        tabg = tabpool.tile([YT * XT, 128], F16, tag=f"tab{g}")
        tabs.append(tabg)

    hwdge = [nc.sync, nc.scalar]
    cnt = {"dma": 0, "cp": 0, "q": 0}

    def eng():
        cnt["dma"] += 1
        return hwdge[cnt["dma"] % 2]

    def ccopy(dst, src):
        cnt["cp"] += 1
        if cnt["cp"] % 2 == 0:
            nc.vector.tensor_copy(dst, src)
        else:
            nc.scalar.copy(dst, src)

    # ---------------- phase 1: repack xs -> table[g], one y-block ----------
    def build_block(g, yb):
        y0 = yb * YB
        rows = min(YB, YS - y0)
        ts = []
        for r in range(4):
            # one DMA for all 8 channels of row-shift r: t[p, c*179 + x]
            t = tpool.tile([128, 8 * 179], F32, tag=f"xsb{r}")
            eng().dma_start(
                bass.AP(t.tensor, t.offset, [[t.ap[0][0], rows], [1, 8 * 179]]),
                bass.AP(xs, (y0 + r) * XS + 128 * g,
                        [[XS, rows], [(YS + 4) * XS, 8], [1, 179]]))
            ts.append(t)
        for h in range(2):
            tg2 = tg2pool.tile([128, XH * 128], F16, tag=f"tg2{h}")
            for r in range(4):
                # merged interleave+shift+cast:
                # tg2[p, xu*128 + s*32 + r*8 + c] = t[p, c*179 + XH*h + xu + s]
                dst = bass.AP(tg2.tensor, tg2.offset + r * 8,
                              [[tg2.ap[0][0], rows], [32, 4], [128, XH], [1, 8]])
                srcap = bass.AP(ts[r].tensor, ts[r].offset + XH * h,
                                [[ts[r].ap[0][0], rows], [1, 4], [1, XH], [179, 8]])
                ccopy(dst, srcap)
            # one HWDGE DMA, contiguous 22.5KB per row on both sides
            dsta = bass.AP(tabs[g].tensor,
                           tabs[g].offset + (y0 * XT + h * XH) * 128,
                           [[XT * 128, rows], [1, XH * 128]])
            eng().dma_start(dsta, tg2[:rows, :])

    # ---------------- phase 2: per super-tile weights+idx, gather+combine --
    def cubic(t, tag, outdt, opool):
        # returns w0..w3 tiles [128, SJW] in outdt; all-DVE (no cross-engine
        # handoffs); scratch tags shared between calls (sequential use).
        TS = nc.vector.tensor_scalar
        TT = nc.vector.tensor_tensor
        s0 = wrk.tile([128, SJW], F32, tag="c_s0")
        TS(s0[:], t[:], 1.0, None, op0=OP.add)
        w0f = wrk.tile([128, SJW], F32, tag="c_w0f")
        TS(w0f[:], s0[:], A, -5.0 * A, op0=OP.mult, op1=OP.add)
        TT(w0f[:], w0f[:], s0[:], op=OP.mult)
        TS(w0f[:], w0f[:], 8.0 * A, None, op0=OP.add)
        TT(w0f[:], w0f[:], s0[:], op=OP.mult)
        w0 = opool.tile([128, SJW], outdt, tag=f"w0{tag}")
        TS(w0[:], w0f[:], -4.0 * A, None, op0=OP.add)
        # w1
        w1f = wrk.tile([128, SJW], F32, tag="c_w1f")
        TS(w1f[:], t[:], A + 2.0, -(A + 3.0), op0=OP.mult, op1=OP.add)
        TT(w1f[:], w1f[:], t[:], op=OP.mult)
        TT(w1f[:], w1f[:], t[:], op=OP.mult)
        w1 = opool.tile([128, SJW], outdt, tag=f"w1{tag}")
        TS(w1[:], w1f[:], 1.0, None, op0=OP.add)
        # w2: u = 1 - t
        u = wrk.tile([128, SJW], F32, tag="c_u")
        TS(u[:], t[:], -1.0, 1.0, op0=OP.mult, op1=OP.add)
        w2f = wrk.tile([128, SJW], F32, tag="c_w2f")
        TS(w2f[:], u[:], A + 2.0, -(A + 3.0), op0=OP.mult, op1=OP.add)
        TT(w2f[:], w2f[:], u[:], op=OP.mult)
        TT(w2f[:], w2f[:], u[:], op=OP.mult)
        w2 = opool.tile([128, SJW], outdt, tag=f"w2{tag}")
        TS(w2[:], w2f[:], 1.0, None, op0=OP.add)
        # w3 = 1 - w0 - w1 - w2 (in f32 then cast)
        w3f = wrk.tile([128, SJW], F32, tag="c_w3f")
        TT(w3f[:], w0[:], w1[:], op=OP.add)
        TT(w3f[:], w3f[:], w2[:], op=OP.add)
        w3 = opool.tile([128, SJW], outdt, tag=f"w3{tag}")
        TS(w3[:], w3f[:], -1.0, 1.0, op0=OP.mult, op1=OP.add)
        return [w0, w1, w2, w3]

    def floorpair(v, tag):
        # vi/co scratch shared between calls; vf/fr persist per-dir
        vi = wrk.tile([128, SJW], I32, tag="f_vi")
        nc.vector.tensor_copy(vi[:], v[:])
        vf = wrk.tile([128, SJW], F32, tag=f"vf{tag}")
        nc.vector.tensor_copy(vf[:], vi[:])
        co = wrk.tile([128, SJW], F32, tag="f_co")
        nc.vector.tensor_tensor(co[:], vf[:], v[:], op=OP.is_gt)
        nc.vector.tensor_tensor(vf[:], vf[:], co[:], op=OP.subtract)
        fr = wrk.tile([128, SJW], F32, tag=f"fr{tag}")
        nc.vector.tensor_tensor(fr[:], v[:], vf[:], op=OP.subtract)
        return vf, fr

    def super_tile(g, s4):
        """Weights + wrapped idx for 512 output cols of row-group g.
        Returns (wxp, wy, C_idx) tiles."""
        IG = g * 128
        jb4 = s4 * SJW
        gt = gridp.tile([128, SJW * 2], F32, tag="gt")
        eng().dma_start(
            gt[:],
            bass.AP(gr, IG * W * 2 + jb4 * 2, [[W * 2, 128], [1, SJW * 2]]))
        gx = bass.AP(gt.tensor, gt.offset, [gt.ap[0], [2, SJW]])
        gy = bass.AP(gt.tensor, gt.offset + 1, [gt.ap[0], [2, SJW]])

        lx = wrk.tile([128, SJW], F32, tag="lx")
        ly = wrk.tile([128, SJW], F32, tag="ly")
        nc.vector.tensor_scalar(lx[:], gx, 1024.0, 1047.5 - IG,
                                op0=OP.mult, op1=OP.add)
        nc.vector.tensor_scalar(ly[:], gy, 1024.0, 1046.5 - jb4,
                                op0=OP.mult, op1=OP.add)
        fx, tx = floorpair(lx, "x")
        fy, ty = floorpair(ly, "y")

        # idxf = fy*XT + fx - 1 (f32, exact)
        idxf = wrk.tile([128, SJW], F32, tag="idxf")
        nc.vector.scalar_tensor_tensor(idxf[:], fy[:], float(XT), fx[:],
                                       op0=OP.mult, op1=OP.add)
        # per-sub-tile rebase to the 186-row gather window, cast to i16
        idx16 = idxs1.tile([128, SJW], I16, tag="idx16")
        for t in range(SJW // 128):
            nc.vector.tensor_scalar(
                bass.AP(idx16.tensor, idx16.offset + t * 128,
                        [[idx16.ap[0][0], 128], [1, 128]]),
                bass.AP(idxf.tensor, idxf.offset + t * 128,
                        [[idxf.ap[0][0], 128], [1, 128]]),
                -1.0 - t * 128.0 * XT, None, op0=OP.add)

        # fold [128, SJW] -> wrapped [16, 8*SJW]: D[p, k*SJW + j] = idx16[16k+p, j]
        D = idxs1.tile([128, 8 * SJW], I16, tag="D")
        for k in range(8):
            src = bass.AP(idx16.tensor,
                          idx16.offset + 16 * k * idx16.ap[0][0],
                          [[idx16.ap[0][0], 16], [1, SJW]])
            dst = bass.AP(D.tensor, D.offset + k * SJW,
                          [[D.ap[0][0], 16], [1, SJW]])
            eng().dma_start(dst, src)
        # interleave: Cw[p, 8j+k] = D[p, k*SJW + j]  (one strided copy)
        Cw = idxp.tile([128, 8 * SJW], I16, tag="Cw")
        nc.vector.tensor_copy(
            bass.AP(Cw.tensor, Cw.offset, [[Cw.ap[0][0], 16], [8, SJW], [1, 8]]),
            bass.AP(D.tensor, D.offset, [[D.ap[0][0], 16], [1, SJW], [SJW, 8]]))
        # replicate to all 8 gpsimd cores
        for rep in range(1, 8):
            src = bass.AP(Cw.tensor, Cw.offset, [[Cw.ap[0][0], 16], [1, 8 * SJW]])
            dst = bass.AP(Cw.tensor, Cw.offset + 16 * rep * Cw.ap[0][0],
                          [[Cw.ap[0][0], 16], [1, 8 * SJW]])
            eng().dma_start(dst, src)

        wx = cubic(tx, "x", F16, wrk)
        wy = cubic(ty, "y", F16, wpp)
        # wxp[j, s] packed s-minor, f16
        wxp = wpp.tile([128, SJW * 4], F16, tag="wxp")
        for s in range(4):
            dst = bass.AP(wxp.tensor, wxp.offset + s, [wxp.ap[0], [4, SJW]])
            nc.scalar.copy(dst, wx[s][:])
        return wxp, wy, Cw

    def half_tile(g, s4, t, h, wxp, wy, Cw):
        IG = g * 128
        jb = s4 * SJW + t * 128 + h * JW2
        ybase = s4 * SJW + t * 128
        # wp_h[j, s, r] = wxp[j, s] * wy_r[j]  (JW2 cols)
        joff = t * 128 + h * JW2
        wp = wpp.tile([128, JW2 * 16], F16, tag="wp")
        for r in range(4):
            dst = bass.AP(wp.tensor, wp.offset + r,
                          [wp.ap[0], [16, JW2], [4, 4]])
            src0 = bass.AP(wxp.tensor, wxp.offset + joff * 4,
                           [wxp.ap[0], [4, JW2], [1, 4]])
            src1 = bass.AP(wy[r].tensor, wy[r].offset + joff,
                           [wy[r].ap[0], [1, JW2], [0, 4]])
            nc.vector.tensor_tensor(dst, src0, src1, op=OP.mult)

        G = gp.tile([128, JW2 * 128], F16, tag="G")
        in_ap = bass.AP(tabs[g].tensor,
                        tabs[g].offset + ybase * XT * 128,
                        [[128, 186 * XT], [1, 128]])
        NSUB = 4096
        for m in range(2):
            idxs = bass.AP(Cw.tensor,
                           Cw.offset + (t * 128 + h * JW2) * 8 + m * (NSUB // 16),
                           [[Cw.ap[0][0], 128], [1, NSUB // 16]])
            q = cnt["q"] % 4
            cnt["q"] += 1
            nc.gpsimd.dma_gather(
                out_ap=bass.AP(G.tensor, G.offset + m * 32 * 128,
                               [[G.ap[0][0], 128], [128, 32], [1, 128]]),
                in_ap=in_ap,
                idxs_ap=idxs,
                num_idxs=NSUB,
                num_idxs_reg=NSUB,
                elem_size=128,
                elem_step=128,
                single_packet=False,
                queue_num=q,
            )

        # combine: P = G * wp (bcast over c), tree-reduce s then r
        src1 = bass.AP(wp.tensor, wp.offset,
                       [wp.ap[0], [16, JW2], [4, 4], [1, 4], [0, 8]])
        src0 = bass.AP(G.tensor, G.offset,
                       [G.ap[0], [128, JW2], [32, 4], [8, 4], [1, 8]])
        nc.vector.tensor_tensor(src0, src0, src1, op=OP.mult)

        def halve(buf, stride, n, tag, npx=JW2):
            o = lp.tile([128, npx * stride * (n // 2)], F16, tag=tag)
            i0 = bass.AP(buf.tensor, buf.offset,
                         [buf.ap[0], [stride * n, npx], [stride * 2, n // 2], [1, stride]])
            i1 = bass.AP(buf.tensor, buf.offset + stride,
                         [buf.ap[0], [stride * n, npx], [stride * 2, n // 2], [1, stride]])
            od = bass.AP(o.tensor, o.offset,
                         [o.ap[0], [stride * (n // 2), npx], [stride, n // 2], [1, stride]])
            nc.vector.tensor_tensor(od, i0, i1, op=OP.add)
            return o

        L1 = halve(G, 32, 4, "L1")
        L2 = halve(L1, 32, 2, "L2")
        L3 = halve(L2, 8, 4, "L3")
        of = outp.tile([128, 8 * JW2], F32, tag="of")
        i0 = bass.AP(L3.tensor, L3.offset, [L3.ap[0], [1, 8], [16, JW2]])
        i1 = bass.AP(L3.tensor, L3.offset + 8, [L3.ap[0], [1, 8], [16, JW2]])
        od = bass.AP(of.tensor, of.offset, [of.ap[0], [JW2, 8], [1, JW2]])
        nc.vector.tensor_tensor(od, i0, i1, op=OP.add)

        # blocked layout [RPC, W//64, C, 64]: 2KB contiguous per partition
        dsto = bass.AP(out, (IG * (W // JW2) + jb // JW2) * C * JW2,
                       [[(W // JW2) * C * JW2, 128], [1, C * JW2]])
        eng().dma_start(dsto, of[:])

    # ---------------- emission schedule --------------------------------
    # Lockstep: table blocks are emitted just-in-time for the j-window each
    # run-super needs (y-prefix dependency); each super's weights/idx are
    # emitted one run-super ahead so they compute during the previous
    # super's gathers/combines.
    run_order = [(0, 0), (1, 0), (0, 1), (1, 1), (0, 2), (1, 2), (0, 3), (1, 3)]
    # blocks of table g needed before running super s4 (window jb4+512+150+pad)
    BLK_NEED = [min(N_YB, (512 * (s + 1) + 150 + PAD + YB - 1) // YB + 1)
                for s in range(4)]
    built = {0: 0, 1: 0}

    def ensure(g, upto):
        while built[g] < upto:
            build_block(g, built[g])
            built[g] += 1

    sups = {}
    ensure(0, BLK_NEED[0])
    sups[(0, 0)] = super_tile(0, 0)
    ensure(1, BLK_NEED[0])
    sups[(1, 0)] = super_tile(1, 0)
    for i, (g, s4) in enumerate(run_order):
        # emit next run-super's weights now (computes during our gathers)
        if i + 1 < len(run_order):
            gn, sn = run_order[i + 1]
            if (gn, sn) not in sups:
                ensure(gn, BLK_NEED[sn])
                sups[(gn, sn)] = super_tile(gn, sn)
        # drip remaining builds of the *other* table between half-tiles
        halves = [(t, h) for t in range(4) for h in range(2)]
        og = 1 - g
        need_more = (N_YB - built[og]) if i >= len(run_order) - 3 else 0
        for k, (t, h) in enumerate(halves):
            half_tile(g, s4, t, h, *sups[(g, s4)])
            if need_more and k % 3 == 2 and built[og] < N_YB:
                build_block(og, built[og])
                built[og] += 1
        del sups[(g, s4)]



_NC_CACHE = None


def kernel(x: np.ndarray, grid: np.ndarray) -> np.ndarray:
    global _NC_CACHE
    if _NC_CACHE is None:
        _NC_CACHE = build_nc()
    nc = _NC_CACHE

    x0 = np.ascontiguousarray(x[0], dtype=np.float32)        # [C, H, W]
    g0 = np.ascontiguousarray(grid[0], dtype=np.float32)     # [H, W, 2]

    in_maps = []
    for k in range(N_CORES):
        I0 = k * RPC
        xsl = np.zeros((C, YS + 4, XS), dtype=np.float32)
        c0 = I0 - PAD
        lo, hi = max(0, c0), min(W, c0 + XS)
        xsl[:, PAD:PAD + H, lo - c0:hi - c0] = x0[:, :, lo:hi]
        grc = np.ascontiguousarray(g0[I0:I0 + RPC]).copy()
        grc[..., 0] -= I0 / 1024.0   # fold per-core x-base into gx
        in_maps.append({"xs": xsl, "gr": grc})

    res = run_bass_kernel_spmd(nc, in_maps, core_ids=list(range(N_CORES)),
                               trace=False)
    global _LAST_EXEC_NS
    _LAST_EXEC_NS = res.exec_time_ns
    out = np.empty((1, C, H, W), dtype=np.float32)
    for k in range(N_CORES):
        blk = res.results[k]["out"]          # [RPC, W//64, C, 64]
        out[0, :, k * RPC:(k + 1) * RPC, :] = (
            blk.transpose(2, 0, 1, 3).reshape(C, RPC, W))
    return out


# revision 17
# speedup vs baseline: 1.3161x; 1.0307x over previous
"""Bicubic grid_sample (transpose-like warp) for Trainium2, 8 NeuronCores.

Strategy: shard output rows across cores (256 rows/core). The warp maps
output (i, j) -> input (y ~ j +- 21, x ~ i +- 21), so each core needs an
x-column slab of the image. On device, repack the slab into a patch table
in DRAM where each 256B unit holds the full 4x4x8ch bicubic patch at
(y0, x0) (fp16). v2: the table is built from a fully s,r-materialized
SBUF staging buffer and written via SWDGE with ~22KB contiguous
descriptors (16-engine spread); indices are folded into the gather's
wrapped 16-partition layout on-chip (no DRAM bounce); weights are
computed on 512-wide super-tiles; row-group 1's table build is emitted
interleaved with row-group 0's gather/combine so they overlap.
"""
import os, sys, types
sys.path.insert(0, "/opt/trn_rl_repo")
import numpy as np

try:  # register NTFF profile hook so BASS_TRACE=1 can measure HW time
    import antenv
    if "antenv.axon_hooks" not in sys.modules:
        from trn_agent_boot.trn_boot import _ntff_profile_via_ctypes
        _h = _ntff_profile_via_ctypes("/opt/axon/libaxon_pjrt.so")
        _m = types.ModuleType("antenv.axon_hooks")
        _m.get_axon_ntff_profile_hook = lambda: _h
        _m.set_axon_ntff_profile_hook = lambda h: None
        sys.modules["antenv.axon_hooks"] = _m
        antenv.axon_hooks = _m
except Exception:
    pass

import concourse.bass as bass
import concourse.bacc as bacc
import concourse.mybir as mybir
import concourse.tile as tile
from concourse import library_config
from concourse.bass_utils import run_bass_kernel_spmd

F32 = mybir.dt.float32
F16 = mybir.dt.float16
I16 = mybir.dt.int16
I32 = mybir.dt.int32
OP = mybir.AluOpType
ACTF = None  # set after import

N_CORES = 8
H = W = 2048
C = 8
RPC = H // N_CORES          # output rows per core = 256
PAD = 24                    # y halo rows on each side
YS = H + 2 * PAD            # 2096 slab rows
XS = 308                    # slab cols: [I0-24, I0+284)
XT = 176                    # table cols per row-group
XH = 88                     # x-half of the table staging buffer
YT = YS + 16                # table rows incl. pad so in_ap window stays in-bounds
SJW = 512                   # super-tile width (weights/idx granularity)
JW2 = 64                    # half-tile width (gather/combine granularity)
A = -0.75                   # bicubic constant
YB = 124                    # y-block rows for table build
N_YB = (YS + YB - 1) // YB  # 17


def build_nc():
    nc = bacc.Bacc("TRN2", target_bir_lowering=False, debug=False,
                   num_devices=N_CORES, num_swdge_queues=4)
    xs = nc.dram_tensor("xs", [C, YS + 4, XS], F32, kind="ExternalInput")
    gr = nc.dram_tensor("gr", [RPC, W, 2], F32, kind="ExternalInput")
    out = nc.dram_tensor("out", [RPC, W // JW2, C, JW2], F32,
                         kind="ExternalOutput")

    with tile.TileContext(nc) as tc:
        nc.gpsimd.load_library(library_config.mlp)
        import contextlib
        with contextlib.ExitStack() as ctx:
            _build_body(ctx, tc, nc, xs, gr, out)
    nc.compile()
    return nc


def _build_body(ctx, tc, nc, xs, gr, out):
    Copy = mybir.ActivationFunctionType.Copy
    tabpool = ctx.enter_context(tc.tile_pool(name="tab", bufs=1, space="DRAM"))
    # phase-1 pools
    tpool = ctx.enter_context(tc.tile_pool(name="t", bufs=1))
    tg2pool = ctx.enter_context(tc.tile_pool(name="tg2", bufs=1))
    # phase-2 pools
    gridp = ctx.enter_context(tc.tile_pool(name="grid", bufs=2))
    wrk = ctx.enter_context(tc.tile_pool(name="wrk", bufs=1))
    wpp = ctx.enter_context(tc.tile_pool(name="wpp", bufs=2))
    idxp = ctx.enter_context(tc.tile_pool(name="idx", bufs=2))
    idxs1 = ctx.enter_context(tc.tile_pool(name="idx1", bufs=1))
    gp = ctx.enter_context(tc.tile_pool(name="g", bufs=2))
    lp = ctx.enter_context(tc.tile_pool(name="l", bufs=1))
    outp = ctx.enter_context(tc.tile_pool(name="out", bufs=2))

    tabs = []
    for g in range(2):
        tabg = tabpool.tile([YT * XT, 128], F16, tag=f"tab{g}")
        tabs.append(tabg)

    hwdge = [nc.sync, nc.scalar]
    cnt = {"dma": 0, "cp": 0, "q": 0}

    def eng():
        cnt["dma"] += 1
        return hwdge[cnt["dma"] % 2]

    def ccopy(dst, src):
        cnt["cp"] += 1
        if cnt["cp"] % 2 == 0:
            nc.vector.tensor_copy(dst, src)
        else:
            nc.scalar.copy(dst, src)

    # ---------------- phase 1: repack xs -> table[g], one y-block ----------
    def build_block(g, yb):
        y0 = yb * YB
        rows = min(YB, YS - y0)
        # one DMA per row-shift r covering all 8 channels
        t4 = tpool.tile([128, 4 * 8 * 179], F32, tag="xsb")
        for r in range(4):
            eng().dma_start(
                bass.AP(t4.tensor, t4.offset + r * 8 * 179,
                        [[t4.ap[0][0], rows], [1, 8 * 179]]),
                bass.AP(xs, (y0 + r) * XS + 128 * g,
                        [[XS, rows], [(YS + 4) * XS, 8], [1, 179]]))
        for h in range(2):
            tg2 = tg2pool.tile([128, XH * 128], F16, tag=f"tg2{h}")
            for r in range(4):
                # merged interleave+shift+cast:
                # tg2[p, xu*128 + s*32 + r*8 + c] = t[p, c*179 + XH*h + xu + s]
                dst = bass.AP(tg2.tensor, tg2.offset + r * 8,
                              [[tg2.ap[0][0], rows], [32, 4], [128, XH], [1, 8]])
                srcap = bass.AP(t4.tensor, t4.offset + r * 8 * 179 + XH * h,
                                [[t4.ap[0][0], rows], [1, 4], [1, XH], [179, 8]])
                ccopy(dst, srcap)
            # one HWDGE DMA, contiguous 22.5KB per row on both sides
            dsta = bass.AP(tabs[g].tensor,
                           tabs[g].offset + (y0 * XT + h * XH) * 128,
                           [[XT * 128, rows], [1, XH * 128]])
            eng().dma_start(dsta, tg2[:rows, :])

    # ---------------- phase 2: per super-tile weights+idx, gather+combine --
    def cubic(t, tag, outdt, opool):
        # returns w0..w3 tiles [128, SJW] in outdt; all-DVE (no cross-engine
        # handoffs); scratch tags shared between calls (sequential use).
        TS = nc.vector.tensor_scalar
        TT = nc.vector.tensor_tensor
        s0 = wrk.tile([128, SJW], F32, tag="c_s0")
        TS(s0[:], t[:], 1.0, None, op0=OP.add)
        w0f = wrk.tile([128, SJW], F32, tag="c_w0f")
        TS(w0f[:], s0[:], A, -5.0 * A, op0=OP.mult, op1=OP.add)
        TT(w0f[:], w0f[:], s0[:], op=OP.mult)
        TS(w0f[:], w0f[:], 8.0 * A, None, op0=OP.add)
        TT(w0f[:], w0f[:], s0[:], op=OP.mult)
        w0 = opool.tile([128, SJW], outdt, tag=f"w0{tag}")
        TS(w0[:], w0f[:], -4.0 * A, None, op0=OP.add)
        # w1
        w1f = wrk.tile([128, SJW], F32, tag="c_w1f")
        TS(w1f[:], t[:], A + 2.0, -(A + 3.0), op0=OP.mult, op1=OP.add)
        TT(w1f[:], w1f[:], t[:], op=OP.mult)
        TT(w1f[:], w1f[:], t[:], op=OP.mult)
        w1 = opool.tile([128, SJW], outdt, tag=f"w1{tag}")
        TS(w1[:], w1f[:], 1.0, None, op0=OP.add)
        # w2: u = 1 - t
        u = wrk.tile([128, SJW], F32, tag="c_u")
        TS(u[:], t[:], -1.0, 1.0, op0=OP.mult, op1=OP.add)
        w2f = wrk.tile([128, SJW], F32, tag="c_w2f")
        TS(w2f[:], u[:], A + 2.0, -(A + 3.0), op0=OP.mult, op1=OP.add)
        TT(w2f[:], w2f[:], u[:], op=OP.mult)
        TT(w2f[:], w2f[:], u[:], op=OP.mult)
        w2 = opool.tile([128, SJW], outdt, tag=f"w2{tag}")
        TS(w2[:], w2f[:], 1.0, None, op0=OP.add)
        # w3 = 1 - w0 - w1 - w2 (in f32 then cast)
        w3f = wrk.tile([128, SJW], F32, tag="c_w3f")
        TT(w3f[:], w0[:], w1[:], op=OP.add)
        TT(w3f[:], w3f[:], w2[:], op=OP.add)
        w3 = opool.tile([128, SJW], outdt, tag=f"w3{tag}")
        TS(w3[:], w3f[:], -1.0, 1.0, op0=OP.mult, op1=OP.add)
        return [w0, w1, w2, w3]

    def floorpair(v, tag):
        # vi/co scratch shared between calls; vf/fr persist per-dir
        vi = wrk.tile([128, SJW], I32, tag="f_vi")
        nc.vector.tensor_copy(vi[:], v[:])
        vf = wrk.tile([128, SJW], F32, tag=f"vf{tag}")
        nc.vector.tensor_copy(vf[:], vi[:])
        co = wrk.tile([128, SJW], F32, tag="f_co")
        nc.vector.tensor_tensor(co[:], vf[:], v[:], op=OP.is_gt)
        nc.vector.tensor_tensor(vf[:], vf[:], co[:], op=OP.subtract)
        fr = wrk.tile([128, SJW], F32, tag=f"fr{tag}")
        nc.vector.tensor_tensor(fr[:], v[:], vf[:], op=OP.subtract)
        return vf, fr

    def super_tile(g, s4):
        """Weights + wrapped idx for 512 output cols of row-group g.
        Returns (wxp, wy, C_idx) tiles."""
        IG = g * 128
        jb4 = s4 * SJW
        gt = gridp.tile([128, SJW * 2], F32, tag="gt")
        eng().dma_start(
            gt[:],
            bass.AP(gr, IG * W * 2 + jb4 * 2, [[W * 2, 128], [1, SJW * 2]]))
        gx = bass.AP(gt.tensor, gt.offset, [gt.ap[0], [2, SJW]])
        gy = bass.AP(gt.tensor, gt.offset + 1, [gt.ap[0], [2, SJW]])

        lx = wrk.tile([128, SJW], F32, tag="lx")
        ly = wrk.tile([128, SJW], F32, tag="ly")
        nc.vector.tensor_scalar(lx[:], gx, 1024.0, 1047.5 - IG,
                                op0=OP.mult, op1=OP.add)
        nc.vector.tensor_scalar(ly[:], gy, 1024.0, 1046.5 - jb4,
                                op0=OP.mult, op1=OP.add)
        fx, tx = floorpair(lx, "x")
        fy, ty = floorpair(ly, "y")

        # idxf = fy*XT + fx - 1 (f32, exact)
        idxf = wrk.tile([128, SJW], F32, tag="idxf")
        nc.vector.scalar_tensor_tensor(idxf[:], fy[:], float(XT), fx[:],
                                       op0=OP.mult, op1=OP.add)
        # per-sub-tile rebase to the 186-row gather window, cast to i16
        idx16 = idxs1.tile([128, SJW], I16, tag="idx16")
        for t in range(SJW // 128):
            nc.vector.tensor_scalar(
                bass.AP(idx16.tensor, idx16.offset + t * 128,
                        [[idx16.ap[0][0], 128], [1, 128]]),
                bass.AP(idxf.tensor, idxf.offset + t * 128,
                        [[idxf.ap[0][0], 128], [1, 128]]),
                -1.0 - t * 128.0 * XT, None, op0=OP.add)

        # fold [128, SJW] -> wrapped [16, 8*SJW]: D[p, k*SJW + j] = idx16[16k+p, j]
        D = idxs1.tile([128, 8 * SJW], I16, tag="D")
        for k in range(8):
            src = bass.AP(idx16.tensor,
                          idx16.offset + 16 * k * idx16.ap[0][0],
                          [[idx16.ap[0][0], 16], [1, SJW]])
            dst = bass.AP(D.tensor, D.offset + k * SJW,
                          [[D.ap[0][0], 16], [1, SJW]])
            eng().dma_start(dst, src)
        # interleave: Cw[p, 8j+k] = D[p, k*SJW + j]  (one strided copy)
        Cw = idxp.tile([128, 8 * SJW], I16, tag="Cw")
        nc.vector.tensor_copy(
            bass.AP(Cw.tensor, Cw.offset, [[Cw.ap[0][0], 16], [8, SJW], [1, 8]]),
            bass.AP(D.tensor, D.offset, [[D.ap[0][0], 16], [1, SJW], [SJW, 8]]))
        # replicate to all 8 gpsimd cores
        for rep in range(1, 8):
            src = bass.AP(Cw.tensor, Cw.offset, [[Cw.ap[0][0], 16], [1, 8 * SJW]])
            dst = bass.AP(Cw.tensor, Cw.offset + 16 * rep * Cw.ap[0][0],
                          [[Cw.ap[0][0], 16], [1, 8 * SJW]])
            eng().dma_start(dst, src)

        wx = cubic(tx, "x", F16, wrk)
        wy = cubic(ty, "y", F16, wpp)
        # wxp[j, s] packed s-minor, f16
        wxp = wpp.tile([128, SJW * 4], F16, tag="wxp")
        for s in range(4):
            dst = bass.AP(wxp.tensor, wxp.offset + s, [wxp.ap[0], [4, SJW]])
            nc.scalar.copy(dst, wx[s][:])
        return wxp, wy, Cw

    def half_tile(g, s4, t, h, wxp, wy, Cw):
        IG = g * 128
        jb = s4 * SJW + t * 128 + h * JW2
        ybase = s4 * SJW + t * 128
        # wp_h[j, s, r] = wxp[j, s] * wy_r[j]  (JW2 cols)
        joff = t * 128 + h * JW2
        wp = wpp.tile([128, JW2 * 16], F16, tag="wp")
        for r in range(4):
            dst = bass.AP(wp.tensor, wp.offset + r,
                          [wp.ap[0], [16, JW2], [4, 4]])
            src0 = bass.AP(wxp.tensor, wxp.offset + joff * 4,
                           [wxp.ap[0], [4, JW2], [1, 4]])
            src1 = bass.AP(wy[r].tensor, wy[r].offset + joff,
                           [wy[r].ap[0], [1, JW2], [0, 4]])
            nc.vector.tensor_tensor(dst, src0, src1, op=OP.mult)

        G = gp.tile([128, JW2 * 128], F16, tag="G")
        in_ap = bass.AP(tabs[g].tensor,
                        tabs[g].offset + ybase * XT * 128,
                        [[128, 186 * XT], [1, 128]])
        NSUB = 2048
        src1h = []
        for m in range(4):
            idxs = bass.AP(Cw.tensor,
                           Cw.offset + (t * 128 + h * JW2) * 8 + m * (NSUB // 16),
                           [[Cw.ap[0][0], 128], [1, NSUB // 16]])
            q = cnt["q"] % 4
            cnt["q"] += 1
            nc.gpsimd.dma_gather(
                out_ap=bass.AP(G.tensor, G.offset + m * 16 * 128,
                               [[G.ap[0][0], 128], [128, 16], [1, 128]]),
                in_ap=in_ap,
                idxs_ap=idxs,
                num_idxs=NSUB,
                num_idxs_reg=NSUB,
                elem_size=128,
                elem_step=128,
                single_packet=False,
                queue_num=q,
            )

        # combine: P = G * wp (bcast over c) in two halves (overlaps gathers)
        for m in range(2):
            src1 = bass.AP(wp.tensor, wp.offset + m * 32 * 16,
                           [wp.ap[0], [16, JW2 // 2], [4, 4], [1, 4], [0, 8]])
            src0 = bass.AP(G.tensor, G.offset + m * 32 * 128,
                           [G.ap[0], [128, JW2 // 2], [32, 4], [8, 4], [1, 8]])
            nc.vector.tensor_tensor(src0, src0, src1, op=OP.mult)

        def halve(buf, stride, n, tag, npx=JW2):
            o = lp.tile([128, npx * stride * (n // 2)], F16, tag=tag)
            i0 = bass.AP(buf.tensor, buf.offset,
                         [buf.ap[0], [stride * n, npx], [stride * 2, n // 2], [1, stride]])
            i1 = bass.AP(buf.tensor, buf.offset + stride,
                         [buf.ap[0], [stride * n, npx], [stride * 2, n // 2], [1, stride]])
            od = bass.AP(o.tensor, o.offset,
                         [o.ap[0], [stride * (n // 2), npx], [stride, n // 2], [1, stride]])
            nc.vector.tensor_tensor(od, i0, i1, op=OP.add)
            return o

        L1 = halve(G, 32, 4, "L1")
        L2 = halve(L1, 32, 2, "L2")
        L3 = halve(L2, 8, 4, "L3")
        of = outp.tile([128, 8 * JW2], F32, tag="of")
        i0 = bass.AP(L3.tensor, L3.offset, [L3.ap[0], [1, 8], [16, JW2]])
        i1 = bass.AP(L3.tensor, L3.offset + 8, [L3.ap[0], [1, 8], [16, JW2]])
        od = bass.AP(of.tensor, of.offset, [of.ap[0], [JW2, 8], [1, JW2]])
        nc.vector.tensor_tensor(od, i0, i1, op=OP.add)

        # blocked layout [RPC, W//64, C, 64]: 2KB contiguous per partition
        dsto = bass.AP(out, (IG * (W // JW2) + jb // JW2) * C * JW2,
                       [[(W // JW2) * C * JW2, 128], [1, C * JW2]])
        eng().dma_start(dsto, of[:])

    # ---------------- emission schedule --------------------------------
    # Lockstep: table blocks are emitted just-in-time for the j-window each
    # run-super needs (y-prefix dependency); each super's weights/idx are
    # emitted one run-super ahead so they compute during the previous
    # super's gathers/combines.
    run_order = [(0, 0), (1, 0), (0, 1), (1, 1), (0, 2), (1, 2), (0, 3), (1, 3)]
    # blocks of table g needed before running super s4 (window jb4+512+150+pad)
    BLK_NEED = [min(N_YB, (512 * (s + 1) + 150 + PAD + YB - 1) // YB + 1)
                for s in range(4)]
    built = {0: 0, 1: 0}

    def ensure(g, upto):
        while built[g] < upto:
            build_block(g, built[g])
            built[g] += 1

    sups = {}
    ensure(0, BLK_NEED[0])
    sups[(0, 0)] = super_tile(0, 0)
    ensure(1, BLK_NEED[0])
    sups[(1, 0)] = super_tile(1, 0)
    for i, (g, s4) in enumerate(run_order):
        # emit next run-super's weights now (computes during our gathers)
        if i + 1 < len(run_order):
            gn, sn = run_order[i + 1]
            if (gn, sn) not in sups:
                ensure(gn, BLK_NEED[sn])
                sups[(gn, sn)] = super_tile(gn, sn)
        # drip remaining builds of the *other* table between half-tiles
        halves = [(t, h) for t in range(4) for h in range(2)]
        og = 1 - g
        need_more = (N_YB - built[og]) if i >= len(run_order) - 3 else 0
        for k, (t, h) in enumerate(halves):
            half_tile(g, s4, t, h, *sups[(g, s4)])
            if need_more and k % 3 == 2 and built[og] < N_YB:
                build_block(og, built[og])
                built[og] += 1
        del sups[(g, s4)]



_NC_CACHE = None


def kernel(x: np.ndarray, grid: np.ndarray) -> np.ndarray:
    global _NC_CACHE
    if _NC_CACHE is None:
        _NC_CACHE = build_nc()
    nc = _NC_CACHE

    x0 = np.ascontiguousarray(x[0], dtype=np.float32)        # [C, H, W]
    g0 = np.ascontiguousarray(grid[0], dtype=np.float32)     # [H, W, 2]

    in_maps = []
    for k in range(N_CORES):
        I0 = k * RPC
        xsl = np.zeros((C, YS + 4, XS), dtype=np.float32)
        c0 = I0 - PAD
        lo, hi = max(0, c0), min(W, c0 + XS)
        xsl[:, PAD:PAD + H, lo - c0:hi - c0] = x0[:, :, lo:hi]
        grc = np.ascontiguousarray(g0[I0:I0 + RPC]).copy()
        grc[..., 0] -= I0 / 1024.0   # fold per-core x-base into gx
        in_maps.append({"xs": xsl, "gr": grc})

    res = run_bass_kernel_spmd(nc, in_maps, core_ids=list(range(N_CORES)),
                               trace=False)
    global _LAST_EXEC_NS
    _LAST_EXEC_NS = res.exec_time_ns
    out = np.empty((1, C, H, W), dtype=np.float32)
    for k in range(N_CORES):
        blk = res.results[k]["out"]          # [RPC, W//64, C, 64]
        out[0, :, k * RPC:(k + 1) * RPC, :] = (
            blk.transpose(2, 0, 1, 3).reshape(C, RPC, W))
    return out
